# revision 1
# baseline (speedup 1.0000x reference)
"""Multi-head attention (B=4, S=2048, H=1024, NH=16) on 8 TRN2 NeuronCores.

Sharding: data-parallel over batch (4) x tensor-parallel over heads (2 groups
of 8 heads). Core c handles batch c//2, head-group c%2 (features 512*(c%2)..).
The host pre-transposes x to x^T [H, S] and W to W^T [H, DG] (bf16).

Per-core kernel (layout chosen so every matmul streams the minimum number of
moving rows and the ScalarE exp stream - the binding resource at ~266us -
never waits on layout shuffles):
  1. Projections: Q^T, K^T per head-pair in [feature, token] layout (bf16),
     V in [token, feature] with a ones column per (pair, head, kt-chunk).
     One batched 3D-AP DMA per input block; pair-0 weight slices load first
     so the first score chunk unblocks ~15us in; K projections for later
     token blocks precede all V/Q work since attention qb=0 walks every
     kt-chunk, while PV trails the exp stream by the 33-deep pt ring and
     Q-tb_i is only needed ~66us/qb later.
  2. Attention per head-pair p (2 heads = 128 features), per 512-token
     q-block, per 128-token kt-chunk:
       - two row-tiled QK^T matmuls produce S^T [128 kt, 512 q] per head,
       - one ScalarE activation does exp(S^T * 1/8 + mask) for both heads
         (PSUM -> SBUF bf16; the mask enters as the per-partition bias),
       - PV runs in the [q, d] orientation: stationary = P^T slice
         [128 kt, 128 q], moving = [V | ones] [128 kt, 65]; each matmul
         streams only 65 output rows (vs 512 in the [d, q] orientation),
         accumulating ctx[q, d] + sumexp[q] over the 16 kt-chunks. The four
         q-chunk regions share one PSUM bank per head, so only the bank's
         first matmul sets start=True (start pends the whole 2KB zero
         region; sibling regions accumulate onto pending-zero bytes).
  3. ctx + sumexp stage through SBUF and DMA out in [token, feature] order;
     the host only normalizes (ctx / sumexp) and concatenates.
A short warm-up matmul run keeps the PE p-state at full clock through the
initial DMA era.
"""

from contextlib import ExitStack

import numpy as np

import concourse.mybir as mybir
import concourse.tile as tile
from concourse import bacc
from concourse.bass_utils import run_bass_kernel_spmd

B, S, H, NH, HD = 4, 2048, 1024, 16, 64
NCORES = 8
DP, TP = 4, 2            # batch-parallel x head-group-parallel
HG = NH // TP            # 8 heads per core
DG = HG * HD             # 512 features per core
NPAIR = HG // 2          # 4 head pairs (128 features each)
CCH = H // 128           # 8 contraction chunks for projections
TB = S // 512            # 4 token blocks of 512
TCH = S // 128           # 16 token chunks of 128
QB = S // 512            # 4 q-blocks of 512
F32 = mybir.dt.float32
F32R = mybir.dt.float32r
BF16 = mybir.dt.bfloat16

_CACHED = None
LAST_RESULTS = None   # BassKernelResults of the most recent run (for test.py)
TRACE = False         # set True (or BASS_KERNEL_TRACE=1) to profile the run


def _build_core_program(repeat=1):
    nc = bacc.Bacc(
        "TRN2", target_bir_lowering=False, debug=False, enable_asserts=False
    )

    xqT = nc.declare_dram_parameter("xqT", [H, S], BF16, isOutput=False)
    xkT = nc.declare_dram_parameter("xkT", [H, S], BF16, isOutput=False)
    xvT = nc.declare_dram_parameter("xvT", [H, S], BF16, isOutput=False)
    wqT = nc.declare_dram_parameter("wqT", [H, DG], BF16, isOutput=False)
    wkT = nc.declare_dram_parameter("wkT", [H, DG], BF16, isOutput=False)
    wvT = nc.declare_dram_parameter("wvT", [H, DG], BF16, isOutput=False)
    # bq | bk | mask packed as one [128, NPAIR+NPAIR+TCH] f32 tensor
    cst = nc.declare_dram_parameter("cst", [128, 2 * NPAIR + TCH], F32,
                                    isOutput=False)
    bv = nc.declare_dram_parameter("bv", [128, DG], BF16, isOutput=False)
    # rows: ((pair*2 + head)*QB + qb)*128 + q_local; cols: qc*65 + (d | sumexp)
    out = nc.declare_dram_parameter("out", [NPAIR * 2 * QB * 128, 260], F32,
                                    isOutput=True)

    with tile.TileContext(nc) as tc:
        for _ in range(repeat):
            _emit(tc, nc, xqT, xkT, xvT, wqT, wkT, wvT, cst, bv, out)

    nc.compile()
    return nc


def _emit(tc, nc, xqT, xkT, xvT, wqT, wkT, wvT, cst, bv, out):
    Exp = mybir.ActivationFunctionType.Exp

    pools = ExitStack()
    const = pools.enter_context(tc.tile_pool(name="const", bufs=1))
    persist = pools.enter_context(tc.tile_pool(name="persist", bufs=1))
    xpool = pools.enter_context(tc.tile_pool(name="xpool", bufs=3))
    work = pools.enter_context(tc.tile_pool(name="work", bufs=6))
    # PSUM: sc 2x[128,1024] (4 banks) + proj 2x[128,512] (2 banks)
    #       + ctxA/ctxB [128,260] (2 banks) = 8 banks exactly
    psum = pools.enter_context(tc.tile_pool(name="psum", bufs=1, space="PSUM"))

    # ---- constants / weights ----
    cst_sb = const.tile([128, 2 * NPAIR + TCH], F32, tag="cst")
    bq_sb = cst_sb[:, 0:NPAIR]
    bk_sb = cst_sb[:, NPAIR : 2 * NPAIR]
    mask_sb = cst_sb[:, 2 * NPAIR : 2 * NPAIR + TCH]
    # V bias pre-replicated across partitions by the host so the V
    # drain-copy can add it on the DVE instead of 16 PE ones-matmuls
    bv_rep = const.tile([128, DG], BF16, tag="bv_rep")

    def load_consts():
        nc.sync.dma_start(cst_sb[:], cst[:])

    def load_bv():
        nc.sync.dma_start(bv_rep[:], bv[:])

    # PE p-state warm-up: a run of dependency-free matmuls on a zeroed tile
    # keeps the PE continuously busy through the initial DMA era so the real
    # projections start at full clock (their results are never read)
    def warmup(n):
        wz = xpool.tile([128, 512], BF16, tag="warm", name="wz", bufs=1)
        nc.gpsimd.memset(wz[:], 0.0)
        for i in range(n):
            ps = psum.tile([128, 512], F32, tag="mmp", bufs=2, name="ps")
            nc.tensor.matmul(ps[:, 0:512], wz[:, 0:128], wz[:], start=True,
                             stop=True)

    # weights as [128, cch*DG]: slice (cch, pair) at cols cch*DG + 128*p;
    # one batched 3D-AP DMA per weight matrix (HWDGE overhead is serial)
    w_sb = {}
    w_dram = {"k": wkT, "q": wqT, "v": wvT}

    def _w_views(name):
        if name not in w_sb:
            w_sb[name] = const.tile(
                [128, CCH * DG], BF16, tag=f"w{name}", name=f"w{name}"
            )
        w = w_sb[name]
        wv4 = w[:].rearrange("p (c g d) -> p c g d", c=CCH, g=NPAIR)
        sv4 = w_dram[name][:].rearrange("(c p) (g d) -> p c g d", p=128,
                                        g=NPAIR)
        return wv4, sv4

    def load_w_pair0(name):
        # pair-0 slice only: 4x less data ahead of the first Q/K unit
        wv4, sv4 = _w_views(name)
        nc.sync.dma_start(wv4[:, :, 0, :], sv4[:, :, 0, :])

    def load_w_rest(name):
        wv4, sv4 = _w_views(name)
        nc.sync.dma_start(wv4[:, :, 1:, :], sv4[:, :, 1:, :])

    def load_w(name):
        if name in w_sb:
            return w_sb[name]
        _w_views(name)
        nc.sync.dma_start(
            w_sb[name][:].rearrange("p (c d) -> p c d", c=CCH),
            w_dram[name][:].rearrange("(c p) d -> p c d", p=128),
        )
        return w_sb[name]

    # ---- persistent activations ----
    # Q^T, K^T per pair: [128 features, S tokens] (bf16: the extra rounding
    # costs ~0.2% relative error, well within budget, and halves SBUF)
    qt_sb = [
        persist.tile([128, S], BF16, tag=f"qt{p}", name=f"qt{p}")
        for p in range(NPAIR)
    ]
    kt_sb = [
        persist.tile([128, S], BF16, tag=f"kt{p}", name=f"kt{p}")
        for p in range(NPAIR)
    ]
    # V with a ones column per (pair, head, kt-chunk): col layout
    # p*(TCH*2*65) + (c*2+h)*65 + d, d in 0..64 where col 64 is ones
    v_sb = persist.tile([128, NPAIR * TCH * 2 * 65], BF16, tag="v")
    v_r = v_sb[:].rearrange("t (p c2 d) -> t p c2 d", p=NPAIR, c2=TCH * 2)
    nc.gpsimd.memset(v_r[:, :, :, 64:65], 1.0)

    # resident x_q^T [128, cch*S] bf16 (32KB/partition) so Q projection for
    # one pair needs no DMA and can interleave with attention
    xq_res = persist.tile([128, CCH * S], BF16, tag="xq_res")
    xq_res_v = xq_res[:].rearrange("p (c t) -> p c t", c=CCH)

    def load_xq_res_tb(tb):
        t_sl = slice(512 * tb, 512 * (tb + 1))
        nc.sync.dma_start(
            xq_res_v[:, :, t_sl],
            xqT[:].rearrange("(c p) t -> p c t", p=128)[:, :, t_sl],
        )

    # one staged x tile [128, cch*512] per (matrix, token block), one DMA each
    def load_x_stage(xT, key, tb, bufs=2):
        xt = xpool.tile(
            [128, CCH * 512], BF16, tag=f"x{key}", name=f"x{key}", bufs=bufs
        )
        nc.sync.dma_start(
            xt[:].rearrange("p (c t) -> p c t", c=CCH),
            xT[:].rearrange("(c p) t -> p c t", p=128)[
                :, :, 512 * tb : 512 * (tb + 1)
            ],
        )
        return xt

    # ---- projection building blocks ----
    def qk_proj_tb(xt, wkey, dst, bias_sb, prs, tb):
        # [feature, token] output for the given pairs, one token block
        for p in prs:
            ps = psum.tile([128, 512], F32, tag="mmp", bufs=2, name="ps")
            for c in range(CCH):
                nc.tensor.matmul(
                    ps[:, 0:512],
                    (w_sb[wkey][:, c * DG + 128 * p : c * DG + 128 * (p + 1)]),
                    (xt[:, 512 * c : 512 * (c + 1)]),
                    start=(c == 0),
                    stop=(c == CCH - 1),
                )
            nc.vector.tensor_scalar_add(
                dst[p][:, 512 * tb : 512 * (tb + 1)],
                ps[:, 0:512],
                bias_sb[:, p : p + 1],
            )

    def v_proj_tb(xt, tb):
        # V[token, feature], all pairs, one token block of x_v^T
        for j in range(4):
            c = 4 * tb + j
            ps = psum.tile([128, 512], F32, tag="mmp", bufs=2, name="ps")
            for cc in range(CCH):
                nc.tensor.matmul(
                    ps[:, 0:512],
                    (xt[:, 512 * cc + 128 * j : 512 * cc + 128 * (j + 1)]),
                    (w_sb["v"][:, cc * DG : (cc + 1) * DG]),
                    start=(cc == 0),
                    stop=(cc == CCH - 1),
                )
            nc.vector.tensor_add(
                v_r[:, :, 2 * c : 2 * c + 2, 0:64],
                ps[:, 0:512].rearrange("t (p h d) -> t p h d", p=NPAIR, h=2),
                bv_rep[:].rearrange("t (p h d) -> t p h d", p=NPAIR, h=2),
            )

    def q_proj_tb(p, tb):
        load_w("q")
        ps = psum.tile([128, 512], F32, tag="mmp", bufs=2, name="ps")
        for c in range(CCH):
            nc.tensor.matmul(
                ps[:, 0:512],
                (w_sb["q"][:, c * DG + 128 * p : c * DG + 128 * (p + 1)]),
                (xq_res[:, c * S + 512 * tb : c * S + 512 * (tb + 1)]),
                start=(c == 0),
                stop=(c == CCH - 1),
            )
        nc.vector.tensor_scalar_add(
            qt_sb[p][:, 512 * tb : 512 * (tb + 1)],
            ps[:, 0:512],
            bq_sb[:, p : p + 1],
        )

    # ---- attention for one (pair, q-block) ----
    def attention_block(p, qb):
        q_sl = slice(512 * qb, 512 * (qb + 1))
        # ctx accumulators in [q, d] orientation: one bank per head holding
        # 4 q-chunks x (64 ctx + 1 sumexp) columns
        ctx_ps = [
            psum.tile([128, 260], F32, tag=f"ctx{h}", name=f"ctx{h}")
            for h in range(2)
        ]
        for c in range(TCH):
            kt_sl = slice(128 * c, 128 * (c + 1))
            sc = psum.tile([128, 1024], F32, tag="mm", bufs=2)
            for h in (0, 1):
                hp = slice(64 * h, 64 * (h + 1))
                nc.tensor.matmul(
                    sc[:, 512 * h : 512 * (h + 1)],
                    (kt_sb[p][hp, kt_sl]),
                    (qt_sb[p][hp, q_sl]),
                    start=True,
                    stop=True,
                )
            pt = work.tile([128, 1024], BF16, tag="pt", bufs=32)
            nc.scalar.activation(
                pt[:], sc[:], Exp, bias=mask_sb[:, c : c + 1], scale=0.125
            )
            for h in range(2):
                for qc in range(4):
                    # start=True pends-to-zero the WHOLE 2KB psum bank
                    # (ZERO_REGION_SIZE), so only the bank's first matmul may
                    # set it; the other qc regions accumulate onto
                    # pending-zero bytes, which read as zero.
                    nc.tensor.matmul(
                        ctx_ps[h][:, 65 * qc : 65 * qc + 65],
                        (pt[:, 512 * h + 128 * qc : 512 * h + 128 * (qc + 1)]),
                        (v_r[:, p, 2 * c + h, :]),
                        start=(c == 0 and qc == 0),
                        stop=(c == TCH - 1),
                        skip_group_check=True,
                    )
        # ctx + sumexp: PSUM -> SBUF staging (DVE) -> DRAM; host normalizes
        for h in range(2):
            cs = work.tile([128, 260], F32, tag="cs", name="cs", bufs=3)
            nc.vector.tensor_copy(out=cs[:], in_=ctx_ps[h][:])
            r0 = ((p * 2 + h) * QB + qb) * 128
            nc.sync.dma_start(out[r0 : r0 + 128, :], cs[:])

    # ---- phase order ----
    # DMA + emission order front-loads what attention (p0, qb=0) chunk 0
    # needs: W_k, x_k tb0, consts, W_q, x_q tb0, then K/Q projections
    # pair-interleaved so pair 0's score chain unblocks first. Attention is
    # emitted inside tc.high_priority() so the scheduler treats it as
    # earliest work: each attention chunk fires the moment its K-block /
    # V-chunk / Q-block lands, and the remaining projection matmuls fill the
    # PE whenever attention is waiting on the ScalarE exp chain.
    warmup(18)
    load_w_pair0("k")
    allp = list(range(NPAIR))
    xk0 = load_x_stage(xkT, "k", 0)
    load_consts()
    load_w_pair0("q")
    load_xq_res_tb(0)
    load_w_rest("k")
    load_w_rest("q")
    # pair-interleaved K/Q for tb0 so pair 0's score chain unblocks first
    for p in allp:
        qk_proj_tb(xk0, "k", kt_sb, bk_sb, [p], 0)
        q_proj_tb(p, 0)
    # K for tb1-3 next (DMAs and projections): attention qb=0 walks all 16
    # kt-chunks, so these gate the exp stream. V only feeds PV, which trails
    # the exp stream by the pt ring; Q for tb>=1 is needed one qb-round
    # (~66us of exp) later still.
    xk_rest = [load_x_stage(xkT, "k", tb) for tb in range(1, TB)]
    for tb in range(1, TB):
        qk_proj_tb(xk_rest[tb - 1], "k", kt_sb, bk_sb, allp, tb)
    load_bv()
    load_w("v")
    xv = [load_x_stage(xvT, "v", tb) for tb in range(TB)]
    for tb in range(TB):
        v_proj_tb(xv[tb], tb)
    for tb in range(1, TB):
        load_xq_res_tb(tb)
        for p in allp:
            q_proj_tb(p, tb)
    for p, qb in [(p, qb) for qb in range(QB) for p in allp]:
        with tc.high_priority():
            attention_block(p, qb)

    pools.close()


def make_in_maps(x_q, x_k, x_v, att_mask, W_q, b_q, W_k, b_k, W_v, b_v):
    import ml_dtypes

    f = np.float32
    bf = ml_dtypes.bfloat16
    x_q, x_k, x_v = (np.asarray(a, f) for a in (x_q, x_k, x_v))
    att_mask = np.asarray(att_mask, f)
    W_q, W_k, W_v = (np.asarray(a, f) for a in (W_q, W_k, W_v))
    b_q, b_k, b_v = (np.asarray(a, f) for a in (b_q, b_k, b_v))

    in_maps = []
    for core in range(NCORES):
        b, g = divmod(core, TP)
        fsl = slice(DG * g, DG * (g + 1))
        in_maps.append(
            {
                "xqT": np.ascontiguousarray(x_q[b].T.astype(bf)),
                "xkT": np.ascontiguousarray(x_k[b].T.astype(bf)),
                "xvT": np.ascontiguousarray(x_v[b].T.astype(bf)),
                "wqT": np.ascontiguousarray(W_q[fsl, :].T.astype(bf)),
                "wkT": np.ascontiguousarray(W_k[fsl, :].T.astype(bf)),
                "wvT": np.ascontiguousarray(W_v[fsl, :].T.astype(bf)),
                "cst": np.ascontiguousarray(
                    np.concatenate(
                        [
                            b_q[fsl].reshape(NPAIR, 128).T,
                            b_k[fsl].reshape(NPAIR, 128).T,
                            att_mask[b, 0, 0].reshape(TCH, 128).T,
                        ],
                        axis=1,
                    )
                ),
                "bv": np.ascontiguousarray(
                    np.broadcast_to(b_v[fsl].reshape(1, DG), (128, DG))
                ).astype(bf),
            }
        )
    return in_maps


def kernel(x_q, x_k, x_v, att_mask, W_q, b_q, W_k, b_k, W_v, b_v):
    global _CACHED
    if _CACHED is None:
        _CACHED = _build_core_program()
    nc = _CACHED

    in_maps = make_in_maps(
        x_q, x_k, x_v, att_mask, W_q, b_q, W_k, b_k, W_v, b_v
    )

    import os

    global LAST_RESULTS
    trace = TRACE or os.environ.get("BASS_KERNEL_TRACE", "") == "1"
    try:
        res = run_bass_kernel_spmd(nc, in_maps, list(range(NCORES)), trace=trace)
    except Exception:
        if not trace:
            raise
        # profiling hook unavailable (e.g. trimmed container) - run untraced
        res = run_bass_kernel_spmd(nc, in_maps, list(range(NCORES)))
    LAST_RESULTS = res

    # out rows are ((pair*2+h)*QB + qb)*128 + q_local, cols qc*65 + (d|sumexp);
    # normalize by the sumexp column and reassemble on the host
    full = np.empty((B, S, H), np.float32)
    for core in range(NCORES):
        b, g = divmod(core, TP)
        r = res.results[core]["out"].reshape(NPAIR, 2, QB, 128, 4, 65)
        ctx = r[..., 0:64] / r[..., 64:65]          # [p, h, qb, i, qc, d]
        # q = qb*512 + qc*128 + i ; feature = (p*2+h)*64 + d
        full[b, :, DG * g : DG * (g + 1)] = (
            ctx.transpose(2, 4, 3, 0, 1, 5).reshape(S, DG)
        )
    return full



# revision 26
# speedup vs baseline: 1.0351x; 1.0351x over previous
"""Multi-head attention (B=4, S=2048, H=1024, NH=16) on 8 TRN2 NeuronCores.

Sharding: data-parallel over batch (4) x tensor-parallel over heads (2 groups
of 8 heads). Core c handles batch c//2, head-group c%2 (features 512*(c%2)..).
The host pre-transposes x to x^T [H, S] and W to W^T [H, DG] (bf16).

v2: flat software-pipelined attention stream. Per-core engine budgets are
PE ~247us (projections 82 + scores 109 + PV 55), ScalarE ~267us if it does
every exp, DVE ~44us. The v1 kernel lost ~60us to PE queue-head stalls
(PV(c) emitted right after scores(c) waits on chunk c's exp; the PE wait
queue is in-order) and ran ScalarE as the binding engine. This version:
  1. offloads the exp of 40/256 score chunks to the DVE via a 4-instruction
     bit-trick exp2 (int32-convert Schraudolph + mantissa-domain cubic
     correction, ~0.35% max elem error, HW-validated), leaving ScalarE
     ~224us and DVE ~234us, both under the PE floor;
  2. emits one flat stream over all 512 (pair, qblock, chunk) slots with PV
     lagged 2 slots behind scores (7 for DVE chunks, whose exp chain has
     ~4.6us latency), so the PE never parks on an unready PV; block
     boundaries overlap because the next block's scores are emitted before
     the previous block's PV flush;
  3. weaves projection units (8 accumulating matmuls + 1 DVE drain) into the
     stream as PE filler in dependency order: K/Q for pair 0 and all of V
     front-loaded, later pairs' K/Q (with their x re-stage DMAs) spread
     ahead of first use.
PSUM: scores 2x[128,1024] (4 banks) + proj 2x[128,512] (2) + ctx 2x[128,260]
(2) = 8 banks exactly.
"""

from contextlib import ExitStack

import numpy as np

import concourse.mybir as mybir
import concourse.tile as tile
from concourse import bacc
from concourse.alu_op_type import AluOpType
from concourse.bass_utils import run_bass_kernel_spmd

B, S, H, NH, HD = 4, 2048, 1024, 16, 64
NCORES = 8
DP, TP = 4, 2            # batch-parallel x head-group-parallel
HG = NH // TP            # 8 heads per core
DG = HG * HD             # 512 features per core
NPAIR = HG // 2          # 4 head pairs (128 features each)
CCH = H // 128           # 8 contraction chunks for projections
TB = S // 512            # 4 token blocks of 512
TCH = S // 128           # 16 token chunks of 128
QB = S // 512            # 4 q-blocks of 512
F32 = mybir.dt.float32
I32 = mybir.dt.int32
BF16 = mybir.dt.bfloat16

# --- DVE exp2 constants: exp(s/8) ~= y*(m^2 + s3*m + t3)*m/y-scale with
# y = bitcast_f32(int32(s*A + Bcol)), m = bitcast_f32((y&MANT)|ONE) --------
EXP_S3 = -2.9705181372354152
EXP_T3 = 6.234666527897044
EXP_K = 4.278858943597909
LOG2E = float(np.log2(np.e))
EXP_A = 2.0 ** 23 * LOG2E / 8.0                  # score -> w scale
EXP_B0 = 127.0 - float(np.log2(EXP_K))           # exponent bias
AND_IMM = float(0x007FFFFF)
OR_IMM = float(0x3F800000)

PV_LAG_SC = 2
PV_LAG_DVE = 6
WARMUP_N = 36
PT_BUFS = 16

_CACHED = None
LAST_RESULTS = None   # BassKernelResults of the most recent run (for test.py)
TRACE = False         # set True (or BASS_KERNEL_TRACE=1) to profile the run


def _register_exp2_op():
    """Register the 6-stage EXP2_FIXUP_ANT custom DVE op (idempotent).

    out = y * ((m + imm2) * m + in1),  m = bitcast_f32((bits(y) & s0) | s1)
    With y = bitcast_f32(int32(score*A + B)) this completes the Schraudolph
    exp2 with a mantissa-domain cubic correction (~0.35% max elem error).
    The op registers through dve_ops' own extension points, so the per-NEFF
    uop table, CoreSim reference, and ISA row all flow normally.
    """
    import concourse.dve_ops as dops
    from concourse.dve_spec import Spec, Src0, Src1, C0, C1, C2, Bin, AluOp, \
        lower
    from concourse.dve_uop import DveOpSpec

    if "EXP2_FIXUP_ANT" in dops._SUB_OPCODE_FOR_NAME:
        return next(o for o in dops.OPS if o.name == "EXP2_FIXUP_ANT")

    def _ref(in0, in1, s0, s1, imm2):
        t = np.asarray(in0, np.float32).view(np.int32)
        s0b = np.asarray(s0, np.float32).view(np.int32)
        s1b = np.asarray(s1, np.float32).view(np.int32)
        m = ((t & s0b) | s1b).view(np.float32)
        y = t.view(np.float32)
        return (y * ((m + np.float32(imm2)) * m
                     + np.asarray(in1, np.float32))).astype(np.float32)

    a = Bin(AluOp.BITWISE_AND, Src0, C0)
    m = Bin(AluOp.BITWISE_OR, a, C1)
    spec = Spec(body=Src0 * ((m + C2) * m + Src1), reference=_ref)
    row = max(dops._SUB_OPCODE_FOR_NAME.values()) + 1
    assert row < 0x20
    uops = lower(spec, ver="v3")
    sha = DveOpSpec(name="EXP2_FIXUP_ANT", opcode=row, uops=uops,
                    rd1_en=True).sha("v3")
    op = dops.DveOp("EXP2_FIXUP_ANT", spec, subdim=False,
                    uops_sha={"v3": sha})
    dops.OPS.append(op)
    dops.CUSTOM_DVE_SPECS[op.name] = spec
    dops._SUB_OPCODE_FOR_NAME[op.name] = row
    return op


def _dve_positions(blk):
    # Back-weighted DVE offload (49 chunks): the early region is PE-bound
    # (V/K/Q filler), so ScalarE alone keeps pace there; later regions have
    # little filler and the exp stream binds, so 4 chunks/block move to
    # the DVE.
    if blk < 3:
        return ()
    if blk == 3:
        return (9,)
    return (1, 5, 9, 13)


def _blk_lag(blk):
    # tapered PV lag: deep early (V projections land ~25us in), 2 at steady
    return max(PV_LAG_SC, 10 - blk)


def _build_core_program(repeat=1):
    nc = bacc.Bacc(
        "TRN2", target_bir_lowering=False, debug=False, enable_asserts=False
    )

    xqT = nc.declare_dram_parameter("xqT", [H, S], BF16, isOutput=False)
    xkT = nc.declare_dram_parameter("xkT", [H, S], BF16, isOutput=False)
    xvT = nc.declare_dram_parameter("xvT", [H, S], BF16, isOutput=False)
    wqT = nc.declare_dram_parameter("wqT", [H, DG], BF16, isOutput=False)
    wkT = nc.declare_dram_parameter("wkT", [H, DG], BF16, isOutput=False)
    wvT = nc.declare_dram_parameter("wvT", [H, DG], BF16, isOutput=False)
    # bq | bk | mask | dve_b | dve_a | and-mask | or-mask packed as
    # [128, 2*NPAIR+2*TCH+3] f32
    cst = nc.declare_dram_parameter("cst", [128, 2 * NPAIR + 2 * TCH + 3],
                                    F32, isOutput=False)
    bv = nc.declare_dram_parameter("bv", [128, DG], BF16, isOutput=False)
    # rows: ((pair*2 + head)*QB + qb)*128 + q_local; cols: qc*65 + (d | sumexp)
    out = nc.declare_dram_parameter("out", [NPAIR * 2 * QB * 128, 260], F32,
                                    isOutput=True)

    with tile.TileContext(nc) as tc:
        for _ in range(repeat):
            _emit(tc, nc, xqT, xkT, xvT, wqT, wkT, wvT, cst, bv, out)

    nc.compile()
    return nc


def _emit(tc, nc, xqT, xkT, xvT, wqT, wkT, wvT, cst, bv, out):
    Exp = mybir.ActivationFunctionType.Exp

    pools = ExitStack()
    const = pools.enter_context(tc.tile_pool(name="const", bufs=1))
    persist = pools.enter_context(tc.tile_pool(name="persist", bufs=1))
    xpool = pools.enter_context(tc.tile_pool(name="xpool", bufs=2))
    work = pools.enter_context(tc.tile_pool(name="work", bufs=6))
    psum = pools.enter_context(tc.tile_pool(name="psum", bufs=1, space="PSUM"))

    # ---- constants / weights ----
    cst_sb = const.tile([128, 2 * NPAIR + 2 * TCH + 3], F32, tag="cst")
    bq_sb = cst_sb[:, 0:NPAIR]
    bk_sb = cst_sb[:, NPAIR : 2 * NPAIR]
    mask_sb = cst_sb[:, 2 * NPAIR : 2 * NPAIR + TCH]
    dveb_sb = cst_sb[:, 2 * NPAIR + TCH : 2 * NPAIR + 2 * TCH]
    dvea_sb = cst_sb[:, 2 * NPAIR + 2 * TCH : 2 * NPAIR + 2 * TCH + 1]
    mand_sb = cst_sb[:, 2 * NPAIR + 2 * TCH + 1 : 2 * NPAIR + 2 * TCH + 2]
    morr_sb = cst_sb[:, 2 * NPAIR + 2 * TCH + 2 : 2 * NPAIR + 2 * TCH + 3]
    bv_rep = const.tile([128, DG], BF16, tag="bv_rep")
    t3c_sb = const.tile([128, 1024], F32, tag="t3c")
    nc.gpsimd.memset(t3c_sb[:], float(EXP_T3))
    exp2_op = _register_exp2_op()

    # PE p-state warm-up through the initial DMA era
    def warmup(n):
        wz = xpool.tile([128, 256], BF16, tag="warm", name="wz", bufs=1)
        nc.gpsimd.memset(wz[:], 0.0)
        for i in range(n):
            ps = psum.tile([128, 512], F32, tag="mmp", bufs=2, name="ps")
            nc.tensor.matmul(ps[:, 0:256], wz[:, 0:128], wz[:], start=True,
                             stop=True)

    # weights as [128, cch*DG]; pair-0 slice loads first
    w_sb = {}
    w_dram = {"k": wkT, "q": wqT, "v": wvT}

    def _w_views(name):
        if name not in w_sb:
            w_sb[name] = const.tile(
                [128, CCH * DG], BF16, tag=f"w{name}", name=f"w{name}"
            )
        w = w_sb[name]
        wv4 = w[:].rearrange("p (c g d) -> p c g d", c=CCH, g=NPAIR)
        sv4 = w_dram[name][:].rearrange("(c p) (g d) -> p c g d", p=128,
                                        g=NPAIR)
        return wv4, sv4

    def load_w_pair0(name):
        wv4, sv4 = _w_views(name)
        nc.sync.dma_start(wv4[:, :, 0, :], sv4[:, :, 0, :])

    def load_w_rest(name):
        wv4, sv4 = _w_views(name)
        nc.sync.dma_start(wv4[:, :, 1:, :], sv4[:, :, 1:, :])

    def load_w(name):
        _w_views(name)
        nc.sync.dma_start(
            w_sb[name][:].rearrange("p (c d) -> p c d", c=CCH),
            w_dram[name][:].rearrange("(c p) d -> p c d", p=128),
        )

    # ---- persistent activations ----
    qt_sb = [
        persist.tile([128, S], BF16, tag=f"qt{p}", name=f"qt{p}")
        for p in range(NPAIR)
    ]
    kt_sb = [
        persist.tile([128, S], BF16, tag=f"kt{p}", name=f"kt{p}")
        for p in range(NPAIR)
    ]
    v_sb = persist.tile([128, NPAIR * TCH * 2 * 65], BF16, tag="v")
    v_r = v_sb[:].rearrange("t (p c2 d) -> t p c2 d", p=NPAIR, c2=TCH * 2)
    nc.gpsimd.memset(v_r[:, :, :, 64:65], 1.0)

    # resident x_q^T [128, cch*S] bf16 (32KB/partition)
    xq_res = persist.tile([128, CCH * S], BF16, tag="xq_res")

    def load_xq_res_tb(tb):
        t_sl = slice(512 * tb, 512 * (tb + 1))
        nc.sync.dma_start(
            xq_res[:].rearrange("p (c t) -> p c t", c=CCH)[:, :, t_sl],
            xqT[:].rearrange("(c p) t -> p c t", p=128)[:, :, t_sl],
        )

    def load_x_stage(xT, key, tb):
        xt = xpool.tile(
            [128, CCH * 512], BF16, tag=f"x{key}", name=f"x{key}",
            bufs=4 if key == "k" else 2,
        )
        nc.sync.dma_start(
            xt[:].rearrange("p (c t) -> p c t", c=CCH),
            xT[:].rearrange("(c p) t -> p c t", p=128)[
                :, :, 512 * tb : 512 * (tb + 1)
            ],
        )
        return xt

    # ---- projection units (8 matmuls + 1 DVE drain each) ----
    def kq_unit(xt, wkey, dst, bias_sb, p, tb):
        ps = psum.tile([128, 512], F32, tag="mmp", bufs=2, name="ps")
        for c in range(CCH):
            nc.tensor.matmul(
                ps[:, 0:512],
                (w_sb[wkey][:, c * DG + 128 * p : c * DG + 128 * (p + 1)]),
                (xt[:, 512 * c : 512 * (c + 1)]),
                start=(c == 0),
                stop=(c == CCH - 1),
            )
        nc.vector.tensor_scalar_add(
            dst[p][:, 512 * tb : 512 * (tb + 1)],
            ps[:, 0:512],
            bias_sb[:, p : p + 1],
        )

    def q_unit(p, tb):
        ps = psum.tile([128, 512], F32, tag="mmp", bufs=2, name="ps")
        for c in range(CCH):
            nc.tensor.matmul(
                ps[:, 0:512],
                (w_sb["q"][:, c * DG + 128 * p : c * DG + 128 * (p + 1)]),
                (xq_res[:, c * S + 512 * tb : c * S + 512 * (tb + 1)]),
                start=(c == 0),
                stop=(c == CCH - 1),
            )
        nc.vector.tensor_scalar_add(
            qt_sb[p][:, 512 * tb : 512 * (tb + 1)],
            ps[:, 0:512],
            bq_sb[:, p : p + 1],
        )

    def v_unit(xt, tb, j):
        c = 4 * tb + j
        ps = psum.tile([128, 512], F32, tag="mmp", bufs=2, name="ps")
        for cc in range(CCH):
            nc.tensor.matmul(
                ps[:, 0:512],
                (xt[:, 512 * cc + 128 * j : 512 * cc + 128 * (j + 1)]),
                (w_sb["v"][:, cc * DG : (cc + 1) * DG]),
                start=(cc == 0),
                stop=(cc == CCH - 1),
            )
        nc.vector.tensor_add(
            v_r[:, :, 2 * c : 2 * c + 2, 0:64],
            ps[:, 0:512].rearrange("t (p h d) -> t p h d", p=NPAIR, h=2),
            bv_rep[:].rearrange("t (p h d) -> t p h d", p=NPAIR, h=2),
        )

    # ---- exp paths ----
    def emit_exp_scalar(sc, c):
        pt = work.tile([128, 1024], BF16, tag="pt", bufs=PT_BUFS)
        nc.scalar.activation(
            pt[:], sc[:], Exp, bias=mask_sb[:, c : c + 1], scale=0.125
        )
        return pt

    # DVE exp is itself software-pipelined: stage 1 (the int32 convert)
    # frees the sc PSUM bank within ~1.2us of the scores, so the score /
    # activation stream never throttles on a busy DVE; stages 2-4 run two
    # slots later.
    def emit_exp_dve1(sc, c):
        ti = work.tile([128, 1024], I32, tag="ti", bufs=2)
        nc.vector.tensor_scalar(ti[:], sc[:], dvea_sb[:, 0:1],
                                dveb_sb[:, c : c + 1],
                                AluOpType.mult, AluOpType.add)
        return ti

    def emit_exp_dve2(ti):
        pt = work.tile([128, 1024], BF16, tag="pt", bufs=PT_BUFS)
        nc.vector._custom_dve(exp2_op, out=pt[:], in0=ti[:].bitcast(F32),
                              in1=t3c_sb[:], s0=mand_sb, s1=morr_sb,
                              imm2=float(EXP_S3))
        return pt

    # ---- flat attention pipeline over slots ----
    NBLK = NPAIR * QB

    ctx_of = {}
    started = {}

    def emit_pv(blk, c, pt, stop):
        p, qb = blk // QB, blk % QB
        if blk not in ctx_of:
            ctx_of[blk] = [
                psum.tile([128, 260], F32, tag=f"ctx{h}", name=f"ctx{h}")
                for h in range(2)
            ]
            started[blk] = [False, False]
        for h in range(2):
            first = not started[blk][h]
            started[blk][h] = True
            for qc in range(4):
                nc.tensor.matmul(
                    ctx_of[blk][h][:, 65 * qc : 65 * qc + 65],
                    (pt[:, 512 * h + 128 * qc : 512 * h + 128 * (qc + 1)]),
                    (v_r[:, p, 2 * c + h, :]),
                    start=(first and qc == 0),
                    stop=(stop and qc == 3),
                    skip_group_check=True,
                )

    def emit_drain(blk):
        p, qb = blk // QB, blk % QB
        for h in range(2):
            cs = work.tile([128, 260], F32, tag="cs", name="cs", bufs=3)
            nc.vector.tensor_scalar_add(cs[:], ctx_of[blk][h][:], 0.0)
            r0 = ((p * 2 + h) * QB + qb) * 128
            nc.sync.dma_start(out[r0 : r0 + 128, :], cs[:])
        del ctx_of[blk]
        del started[blk]

    # ---- preamble: warmup + the critical-path DMAs + pair-0 tb0 units ----
    # DMA serializes at ~2.9us/MB on the DMA fabric; the SP HWDGE queue is
    # in-order, so the queue order is exactly the need order: wk-p0, xk0,
    # wq-p0, xq0, cst unblock the first scores; the rest queues behind in
    # first-use order. The xv tb2/tb3 re-stages (which must WAIT on V-unit
    # consumers) go on the Pool engine's SWDGE queue so their waits cannot
    # head-block the SP queue.
    warmup(WARMUP_N)
    load_w_pair0("k")
    xk = [load_x_stage(xkT, "k", 0)]
    load_w_pair0("q")
    load_xq_res_tb(0)
    nc.sync.dma_start(cst_sb[:], cst[:])
    nc.sync.dma_start(bv_rep[:], bv[:])
    kq_unit(xk[0], "k", kt_sb, bk_sb, 0, 0)
    q_unit(0, 0)
    xk.append(load_x_stage(xkT, "k", 1))
    load_w("v")
    xv_t = {0: load_x_stage(xvT, "v", 0)}
    load_xq_res_tb(1)
    xk.append(load_x_stage(xkT, "k", 2))
    xv_t[1] = load_x_stage(xvT, "v", 1)
    xk.append(load_x_stage(xkT, "k", 3))
    load_xq_res_tb(2)
    load_w_rest("k")
    load_w_rest("q")
    load_xq_res_tb(3)

    # ---- filler schedule: slot -> [callables] ----
    filler = {}

    def add_filler(slot, fn):
        filler.setdefault(slot, []).append(fn)

    # dependency-free warmup batches bridge the early DMA-wait region
    # without blocking the in-order PE queue
    add_filler(1, lambda: warmup(5))
    add_filler(3, lambda: warmup(5))

    # K p0 tb1-3 as soon as their x lands
    add_filler(0, lambda: kq_unit(xk[1], "k", kt_sb, bk_sb, 0, 1))
    add_filler(4, lambda: kq_unit(xk[2], "k", kt_sb, bk_sb, 0, 2))
    add_filler(8, lambda: kq_unit(xk[3], "k", kt_sb, bk_sb, 0, 3))

    # V: one unit per slot for slots 5..20; xv tb2/3 re-staged via Pool DMA
    def stage_xv(tb):
        xt = xpool.tile([128, CCH * 512], BF16, tag="xv", name="xv", bufs=2)
        nc.gpsimd.dma_start(
            xt[:].rearrange("p (c t) -> p c t", c=CCH),
            xvT[:].rearrange("(c p) t -> p c t", p=128)[
                :, :, 512 * tb : 512 * (tb + 1)
            ],
        )
        xv_t[tb] = xt

    add_filler(9, lambda: stage_xv(2))
    add_filler(13, lambda: stage_xv(3))
    for tb in range(TB):
        for j in range(4):
            add_filler(5 + 4 * tb + j,
                       lambda tb=tb, j=j: v_unit(xv_t[tb], tb, j))
    # Q(p, qb) spread to just ahead of block (p,qb) at slot 64p+16qb
    for p in range(NPAIR):
        for qb in range(QB):
            if p == 0 and qb == 0:
                continue
            add_filler(64 * p + 16 * qb - 7, lambda p=p, qb=qb: q_unit(p, qb))
    # later pairs: K units (x_k stays resident) ahead of slot 64*p
    for p in range(1, NPAIR):
        base = 64 * p
        for tb in range(TB):
            add_filler(base - 20 + 4 * tb,
                       lambda p=p, tb=tb: kq_unit(xk[tb], "k", kt_sb,
                                                  bk_sb, p, tb))

    # ---- the flat stream ----
    pend = []     # (due_slot, blk, c, ptbox)
    stage2q = []  # (due_slot, ptbox, ti)
    slot = 0
    for blk in range(NBLK):
        p, qb = blk // QB, blk % QB
        q_sl = slice(512 * qb, 512 * (qb + 1))
        dve_cs = _dve_positions(blk)
        for c in range(TCH):
            kt_sl = slice(128 * c, 128 * (c + 1))
            sc = psum.tile([128, 1024], F32, tag="mm", bufs=2)
            for h in (0, 1):
                hp = slice(64 * h, 64 * (h + 1))
                nc.tensor.matmul(
                    sc[:, 512 * h : 512 * (h + 1)],
                    (kt_sb[p][hp, kt_sl]),
                    (qt_sb[p][hp, q_sl]),
                    start=True,
                    stop=True,
                )
            if c in dve_cs:
                ptbox = []
                stage2q.append((slot + 2, ptbox, emit_exp_dve1(sc, c)))
                lag = max(_blk_lag(blk), PV_LAG_DVE)
            else:
                ptbox = [emit_exp_scalar(sc, c)]
                lag = _blk_lag(blk)
            pend.append((slot + lag, blk, c, ptbox))
            pend.sort(key=lambda e: e[0])
            while stage2q and stage2q[0][0] <= slot:
                _, box, ti = stage2q.pop(0)
                box.append(emit_exp_dve2(ti))
            while pend and pend[0][0] <= slot:
                _, b2, c2, box2 = pend.pop(0)
                done_emitting = b2 < blk or (b2 == blk and c == TCH - 1)
                stop = done_emitting and not any(e[1] == b2 for e in pend)
                emit_pv(b2, c2, box2[0], stop)
                if stop:
                    emit_drain(b2)
            for fn in filler.pop(slot, []):
                fn()
            slot += 1
    while stage2q:
        _, box, ti = stage2q.pop(0)
        box.append(emit_exp_dve2(ti))
    while pend:
        _, b2, c2, box2 = pend.pop(0)
        stop = not any(e[1] == b2 for e in pend)
        emit_pv(b2, c2, box2[0], stop)
        if stop:
            emit_drain(b2)
    for sl in sorted(filler):
        for fn in filler.pop(sl):
            fn()

    pools.close()


def make_in_maps(x_q, x_k, x_v, att_mask, W_q, b_q, W_k, b_k, W_v, b_v):
    import ml_dtypes

    f = np.float32
    bf = ml_dtypes.bfloat16
    x_q, x_k, x_v = (np.asarray(a, f) for a in (x_q, x_k, x_v))
    att_mask = np.asarray(att_mask, f)
    W_q, W_k, W_v = (np.asarray(a, f) for a in (W_q, W_k, W_v))
    b_q, b_k, b_v = (np.asarray(a, f) for a in (b_q, b_k, b_v))

    in_maps = []
    for core in range(NCORES):
        b, g = divmod(core, TP)
        fsl = slice(DG * g, DG * (g + 1))
        mask_col = np.ascontiguousarray(
            att_mask[b, 0, 0].reshape(TCH, 128).T)           # [128, TCH]
        dve_b = (2.0 ** 23 * EXP_B0
                 + mask_col.astype(np.float64) * (2.0 ** 23 * LOG2E)
                 ).astype(np.float32)
        dve_a = np.full((128, 1), EXP_A, np.float32)
        m_and = np.full((128, 1), 0x007FFFFF, np.int32).view(np.float32)
        m_orr = np.full((128, 1), 0x3F800000, np.int32).view(np.float32)
        in_maps.append(
            {
                "xqT": np.ascontiguousarray(x_q[b].T.astype(bf)),
                "xkT": np.ascontiguousarray(x_k[b].T.astype(bf)),
                "xvT": np.ascontiguousarray(x_v[b].T.astype(bf)),
                "wqT": np.ascontiguousarray(W_q[fsl, :].T.astype(bf)),
                "wkT": np.ascontiguousarray(W_k[fsl, :].T.astype(bf)),
                "wvT": np.ascontiguousarray(W_v[fsl, :].T.astype(bf)),
                "cst": np.ascontiguousarray(
                    np.concatenate(
                        [
                            b_q[fsl].reshape(NPAIR, 128).T,
                            b_k[fsl].reshape(NPAIR, 128).T,
                            mask_col,
                            dve_b,
                            dve_a,
                            m_and,
                            m_orr,
                        ],
                        axis=1,
                    )
                ),
                "bv": np.ascontiguousarray(
                    np.broadcast_to(b_v[fsl].reshape(1, DG), (128, DG))
                ).astype(bf),
            }
        )
    return in_maps


def kernel(x_q, x_k, x_v, att_mask, W_q, b_q, W_k, b_k, W_v, b_v):
    global _CACHED
    if _CACHED is None:
        _CACHED = _build_core_program()
    nc = _CACHED

    in_maps = make_in_maps(
        x_q, x_k, x_v, att_mask, W_q, b_q, W_k, b_k, W_v, b_v
    )

    import os

    global LAST_RESULTS
    trace = TRACE or os.environ.get("BASS_KERNEL_TRACE", "") == "1"
    try:
        res = run_bass_kernel_spmd(nc, in_maps, list(range(NCORES)), trace=trace)
    except Exception:
        if not trace:
            raise
        res = run_bass_kernel_spmd(nc, in_maps, list(range(NCORES)))
    LAST_RESULTS = res

    full = np.empty((B, S, H), np.float32)
    for core in range(NCORES):
        b, g = divmod(core, TP)
        r = res.results[core]["out"].reshape(NPAIR, 2, QB, 128, 4, 65)
        ctx = r[..., 0:64] / r[..., 64:65]          # [p, h, qb, i, qc, d]
        full[b, :, DG * g : DG * (g + 1)] = (
            ctx.transpose(2, 4, 3, 0, 1, 5).reshape(S, DG)
        )
    return full


# revision 37
# speedup vs baseline: 1.1076x; 1.0701x over previous
"""Multi-head attention (B=4, S=2048, H=1024, NH=16) on 8 TRN2 NeuronCores.

Sharding: data-parallel over batch (4) x tensor-parallel over heads (2 groups
of 8 heads). Core c handles batch c//2, head-group c%2 (features 512*(c%2)..).
The host pre-transposes x to x^T [H, S] and W to W^T [H, DG] (bf16).

v2: flat software-pipelined attention stream. Per-core engine budgets are
PE ~247us (projections 82 + scores 109 + PV 55), ScalarE ~267us if it does
every exp, DVE ~44us. The v1 kernel lost ~60us to PE queue-head stalls
(PV(c) emitted right after scores(c) waits on chunk c's exp; the PE wait
queue is in-order) and ran ScalarE as the binding engine. This version:
  1. offloads the exp of 40/256 score chunks to the DVE via a 4-instruction
     bit-trick exp2 (int32-convert Schraudolph + mantissa-domain cubic
     correction, ~0.35% max elem error, HW-validated), leaving ScalarE
     ~224us and DVE ~234us, both under the PE floor;
  2. emits one flat stream over all 512 (pair, qblock, chunk) slots with PV
     lagged 2 slots behind scores (7 for DVE chunks, whose exp chain has
     ~4.6us latency), so the PE never parks on an unready PV; block
     boundaries overlap because the next block's scores are emitted before
     the previous block's PV flush;
  3. weaves projection units (8 accumulating matmuls + 1 DVE drain) into the
     stream as PE filler in dependency order: K/Q for pair 0 and all of V
     front-loaded, later pairs' K/Q (with their x re-stage DMAs) spread
     ahead of first use.
PSUM: scores 2x[128,1024] (4 banks) + proj 2x[128,512] (2) + ctx 2x[128,260]
(2) = 8 banks exactly.
"""

from contextlib import ExitStack

import numpy as np

import concourse.mybir as mybir
import concourse.tile as tile
from concourse import bacc
from concourse.alu_op_type import AluOpType
from concourse.bass_utils import run_bass_kernel_spmd

B, S, H, NH, HD = 4, 2048, 1024, 16, 64
NCORES = 8
DP, TP = 4, 2            # batch-parallel x head-group-parallel
HG = NH // TP            # 8 heads per core
DG = HG * HD             # 512 features per core
NPAIR = HG // 2          # 4 head pairs (128 features each)
CCH = H // 128           # 8 contraction chunks for projections
TB = S // 512            # 4 token blocks of 512
TCH = S // 128           # 16 token chunks of 128
QB = S // 512            # 4 q-blocks of 512
F32 = mybir.dt.float32
I32 = mybir.dt.int32
BF16 = mybir.dt.bfloat16

# --- DVE exp2 constants: exp(s/8) ~= y*(m^2 + s3*m + t3)*m/y-scale with
# y = bitcast_f32(int32(s*A + Bcol)), m = bitcast_f32((y&MANT)|ONE) --------
EXP_S3 = -2.9705181372354152
EXP_T3 = 6.234666527897044
EXP_K = 4.278858943597909
LOG2E = float(np.log2(np.e))
EXP_A = 2.0 ** 23 * LOG2E / 8.0                  # score -> w scale
EXP_B0 = 127.0 - float(np.log2(EXP_K))           # exponent bias
AND_IMM = float(0x007FFFFF)
OR_IMM = float(0x3F800000)

PV_LAG_SC = 2
PV_LAG_DVE = 6
WARMUP_N = 54
PT_BUFS = 16

_CACHED = None
LAST_RESULTS = None   # BassKernelResults of the most recent run (for test.py)
TRACE = False         # set True (or BASS_KERNEL_TRACE=1) to profile the run


def _register_exp2_op():
    """Register the 6-stage EXP2_FIXUP_ANT custom DVE op (idempotent).

    out = y * ((m + imm2) * m + in1),  m = bitcast_f32((bits(y) & s0) | s1)
    With y = bitcast_f32(int32(score*A + B)) this completes the Schraudolph
    exp2 with a mantissa-domain cubic correction (~0.35% max elem error).
    The op registers through dve_ops' own extension points, so the per-NEFF
    uop table, CoreSim reference, and ISA row all flow normally.
    """
    import concourse.dve_ops as dops
    from concourse.dve_spec import Spec, Src0, Src1, C0, C1, C2, Bin, AluOp, \
        lower
    from concourse.dve_uop import DveOpSpec

    if "EXP2_FIXUP_ANT" in dops._SUB_OPCODE_FOR_NAME:
        return next(o for o in dops.OPS if o.name == "EXP2_FIXUP_ANT")

    def _ref(in0, in1, s0, s1, imm2):
        t = np.asarray(in0, np.float32).view(np.int32)
        s0b = np.asarray(s0, np.float32).view(np.int32)
        s1b = np.asarray(s1, np.float32).view(np.int32)
        m = ((t & s0b) | s1b).view(np.float32)
        y = t.view(np.float32)
        return (y * ((m + np.float32(imm2)) * m
                     + np.asarray(in1, np.float32))).astype(np.float32)

    a = Bin(AluOp.BITWISE_AND, Src0, C0)
    m = Bin(AluOp.BITWISE_OR, a, C1)
    spec = Spec(body=Src0 * ((m + C2) * m + Src1), reference=_ref)
    row = max(dops._SUB_OPCODE_FOR_NAME.values()) + 1
    assert row < 0x20
    uops = lower(spec, ver="v3")
    sha = DveOpSpec(name="EXP2_FIXUP_ANT", opcode=row, uops=uops,
                    rd1_en=True).sha("v3")
    op = dops.DveOp("EXP2_FIXUP_ANT", spec, subdim=False,
                    uops_sha={"v3": sha})
    dops.OPS.append(op)
    dops.CUSTOM_DVE_SPECS[op.name] = spec
    dops._SUB_OPCODE_FOR_NAME[op.name] = row
    return op


def _dve_positions(blk):
    # Back-weighted DVE offload (49 chunks): the early region is PE-bound
    # (V/K/Q filler), so ScalarE alone keeps pace there; later regions have
    # little filler and the exp stream binds, so 4 chunks/block move to
    # the DVE.
    if blk < 3:
        return ()
    if blk == 3:
        return (9,)
    return (1, 5, 9, 13)


def _blk_lag(blk):
    # tapered PV lag: deep early (V projections land ~25us in), 2 at steady
    return max(PV_LAG_SC, 12 - blk)


def _build_core_program(repeat=1):
    nc = bacc.Bacc(
        "TRN2", target_bir_lowering=False, debug=False, enable_asserts=False
    )

    xqT = nc.declare_dram_parameter("xqT", [H, S], BF16, isOutput=False)
    xkT = nc.declare_dram_parameter("xkT", [H, S], BF16, isOutput=False)
    xvT = nc.declare_dram_parameter("xvT", [H, S], BF16, isOutput=False)
    wqT = nc.declare_dram_parameter("wqT", [H, DG], BF16, isOutput=False)
    wkT = nc.declare_dram_parameter("wkT", [H, DG], BF16, isOutput=False)
    wvT = nc.declare_dram_parameter("wvT", [H, DG], BF16, isOutput=False)
    # bq | bk | mask | dve_b | dve_a | and-mask | or-mask packed as
    # [128, 2*NPAIR+2*TCH+3] f32
    cst = nc.declare_dram_parameter("cst", [128, 2 * NPAIR + 2 * TCH + 3],
                                    F32, isOutput=False)
    bv = nc.declare_dram_parameter("bv", [128, DG], BF16, isOutput=False)
    # rows: ((pair*2 + head)*QB + qb)*128 + q_local; cols: qc*65 + (d | sumexp)
    out = nc.declare_dram_parameter("out", [NPAIR * 2 * QB * 128, 260], F32,
                                    isOutput=True)

    with tile.TileContext(nc) as tc:
        for _ in range(repeat):
            _emit(tc, nc, xqT, xkT, xvT, wqT, wkT, wvT, cst, bv, out)

    nc.compile()
    return nc


def _emit(tc, nc, xqT, xkT, xvT, wqT, wkT, wvT, cst, bv, out):
    Exp = mybir.ActivationFunctionType.Exp

    pools = ExitStack()
    const = pools.enter_context(tc.tile_pool(name="const", bufs=1))
    persist = pools.enter_context(tc.tile_pool(name="persist", bufs=1))
    xpool = pools.enter_context(tc.tile_pool(name="xpool", bufs=2))
    work = pools.enter_context(tc.tile_pool(name="work", bufs=6))
    psum = pools.enter_context(tc.tile_pool(name="psum", bufs=1, space="PSUM"))

    # ---- constants / weights ----
    cst_sb = const.tile([128, 2 * NPAIR + 2 * TCH + 3], F32, tag="cst")
    bq_sb = cst_sb[:, 0:NPAIR]
    bk_sb = cst_sb[:, NPAIR : 2 * NPAIR]
    mask_sb = cst_sb[:, 2 * NPAIR : 2 * NPAIR + TCH]
    dveb_sb = cst_sb[:, 2 * NPAIR + TCH : 2 * NPAIR + 2 * TCH]
    dvea_sb = cst_sb[:, 2 * NPAIR + 2 * TCH : 2 * NPAIR + 2 * TCH + 1]
    mand_sb = cst_sb[:, 2 * NPAIR + 2 * TCH + 1 : 2 * NPAIR + 2 * TCH + 2]
    morr_sb = cst_sb[:, 2 * NPAIR + 2 * TCH + 2 : 2 * NPAIR + 2 * TCH + 3]
    bv_rep = const.tile([128, DG], BF16, tag="bv_rep")
    t3c_sb = const.tile([128, 1024], F32, tag="t3c")
    nc.gpsimd.memset(t3c_sb[:], float(EXP_T3))
    exp2_op = _register_exp2_op()

    # PE p-state warm-up through the initial DMA era
    def warmup(n):
        wz = xpool.tile([128, 256], BF16, tag="warm", name="wz", bufs=1)
        nc.gpsimd.memset(wz[:], 0.0)
        for i in range(n):
            ps = psum.tile([128, 512], F32, tag="mmp", bufs=2, name="ps")
            nc.tensor.matmul(ps[:, 0:256], wz[:, 0:128], wz[:], start=True,
                             stop=True)

    # weights as [128, cch*DG]; pair-0 slice loads first
    w_sb = {}
    w_dram = {"k": wkT, "q": wqT, "v": wvT}

    def _w_views(name):
        if name not in w_sb:
            w_sb[name] = const.tile(
                [128, CCH * DG], BF16, tag=f"w{name}", name=f"w{name}"
            )
        w = w_sb[name]
        wv4 = w[:].rearrange("p (c g d) -> p c g d", c=CCH, g=NPAIR)
        sv4 = w_dram[name][:].rearrange("(c p) (g d) -> p c g d", p=128,
                                        g=NPAIR)
        return wv4, sv4

    def load_w_pair0(name):
        wv4, sv4 = _w_views(name)
        nc.sync.dma_start(wv4[:, :, 0, :], sv4[:, :, 0, :])

    def load_w_rest(name):
        wv4, sv4 = _w_views(name)
        nc.sync.dma_start(wv4[:, :, 1:, :], sv4[:, :, 1:, :])

    def load_w(name):
        _w_views(name)
        nc.sync.dma_start(
            w_sb[name][:].rearrange("p (c d) -> p c d", c=CCH),
            w_dram[name][:].rearrange("(c p) d -> p c d", p=128),
        )

    # ---- persistent activations ----
    qt_sb = [
        persist.tile([128, S], BF16, tag=f"qt{p}", name=f"qt{p}")
        for p in range(NPAIR)
    ]
    kt_sb = [
        persist.tile([128, S], BF16, tag=f"kt{p}", name=f"kt{p}")
        for p in range(NPAIR)
    ]
    v_sb = persist.tile([128, NPAIR * TCH * 2 * 65], BF16, tag="v")
    v_r = v_sb[:].rearrange("t (p c2 d) -> t p c2 d", p=NPAIR, c2=TCH * 2)
    nc.gpsimd.memset(v_r[:, :, :, 64:65], 1.0)

    # resident x_q^T [128, cch*S] bf16 (32KB/partition)
    xq_res = persist.tile([128, CCH * S], BF16, tag="xq_res")

    def load_xq_res_tb(tb):
        t_sl = slice(512 * tb, 512 * (tb + 1))
        nc.sync.dma_start(
            xq_res[:].rearrange("p (c t) -> p c t", c=CCH)[:, :, t_sl],
            xqT[:].rearrange("(c p) t -> p c t", p=128)[:, :, t_sl],
        )

    def load_x_stage(xT, key, tb):
        xt = xpool.tile(
            [128, CCH * 512], BF16, tag=f"x{key}", name=f"x{key}",
            bufs=4 if key == "k" else 2,
        )
        nc.sync.dma_start(
            xt[:].rearrange("p (c t) -> p c t", c=CCH),
            xT[:].rearrange("(c p) t -> p c t", p=128)[
                :, :, 512 * tb : 512 * (tb + 1)
            ],
        )
        return xt

    # ---- projection units (8 matmuls + 1 DVE drain each) ----
    def kq_unit(xt, wkey, dst, bias_sb, p, tb):
        ps = psum.tile([128, 512], F32, tag="mmp", bufs=2, name="ps")
        for c in range(CCH):
            nc.tensor.matmul(
                ps[:, 0:512],
                (w_sb[wkey][:, c * DG + 128 * p : c * DG + 128 * (p + 1)]),
                (xt[:, 512 * c : 512 * (c + 1)]),
                start=(c == 0),
                stop=(c == CCH - 1),
            )
        nc.vector.tensor_scalar_add(
            dst[p][:, 512 * tb : 512 * (tb + 1)],
            ps[:, 0:512],
            bias_sb[:, p : p + 1],
        )

    def q_unit(p, tb):
        ps = psum.tile([128, 512], F32, tag="mmp", bufs=2, name="ps")
        for c in range(CCH):
            nc.tensor.matmul(
                ps[:, 0:512],
                (w_sb["q"][:, c * DG + 128 * p : c * DG + 128 * (p + 1)]),
                (xq_res[:, c * S + 512 * tb : c * S + 512 * (tb + 1)]),
                start=(c == 0),
                stop=(c == CCH - 1),
            )
        nc.vector.tensor_scalar_add(
            qt_sb[p][:, 512 * tb : 512 * (tb + 1)],
            ps[:, 0:512],
            bq_sb[:, p : p + 1],
        )

    def v_unit(xt, tb, j):
        c = 4 * tb + j
        ps = psum.tile([128, 512], F32, tag="mmp", bufs=2, name="ps")
        for cc in range(CCH):
            nc.tensor.matmul(
                ps[:, 0:512],
                (xt[:, 512 * cc + 128 * j : 512 * cc + 128 * (j + 1)]),
                (w_sb["v"][:, cc * DG : (cc + 1) * DG]),
                start=(cc == 0),
                stop=(cc == CCH - 1),
            )
        nc.vector.tensor_add(
            v_r[:, :, 2 * c : 2 * c + 2, 0:64],
            ps[:, 0:512].rearrange("t (p h d) -> t p h d", p=NPAIR, h=2),
            bv_rep[:].rearrange("t (p h d) -> t p h d", p=NPAIR, h=2),
        )

    # ---- exp paths ----
    def emit_exp_scalar(sc, c):
        pt = work.tile([128, 1024], BF16, tag="pt", bufs=PT_BUFS)
        nc.scalar.activation(
            pt[:], sc[:], Exp, bias=mask_sb[:, c : c + 1], scale=0.125
        )
        return pt

    # Split-chunk exp: ScalarE takes head 0, the DVE takes head 1 via the
    # pipelined int32-convert (frees its sc half fast) + custom fixup two
    # slots later.
    def emit_exp_split1(sc, c):
        pt = work.tile([128, 1024], BF16, tag="pt", bufs=PT_BUFS)
        ti = work.tile([128, 1024], I32, tag="ti", bufs=2)
        nc.vector.tensor_scalar(ti[:], sc[:], dvea_sb[:, 0:1],
                                dveb_sb[:, c : c + 1],
                                AluOpType.mult, AluOpType.add)
        return pt, ti

    def emit_exp_split1_mmp(schalves, c):
        pt = work.tile([128, 1024], BF16, tag="pt", bufs=PT_BUFS)
        ti = work.tile([128, 1024], I32, tag="ti", bufs=2)
        for h in (0, 1):
            hs = slice(512 * h, 512 * (h + 1))
            nc.vector.tensor_scalar(ti[:, hs], schalves[h][:, 0:512],
                                    dvea_sb[:, 0:1], dveb_sb[:, c : c + 1],
                                    AluOpType.mult, AluOpType.add)
        return pt, ti

    def emit_exp_split2(ti, pt):
        nc.vector._custom_dve(exp2_op, out=pt[:],
                              in0=ti[:].bitcast(F32),
                              in1=t3c_sb[:], s0=mand_sb, s1=morr_sb,
                              imm2=float(EXP_S3))

    # ---- flat attention pipeline over slots ----
    NBLK = NPAIR * QB

    ctx_of = {}
    started = {}

    def emit_pv(blk, c, pt, stop):
        p, qb = blk // QB, blk % QB
        if blk not in ctx_of:
            ctx_of[blk] = [
                psum.tile([128, 260], F32, tag=f"ctx{h}", name=f"ctx{h}")
                for h in range(2)
            ]
            started[blk] = [False, False]
        for h in range(2):
            first = not started[blk][h]
            started[blk][h] = True
            for qc in range(4):
                nc.tensor.matmul(
                    ctx_of[blk][h][:, 65 * qc : 65 * qc + 65],
                    (pt[:, 512 * h + 128 * qc : 512 * h + 128 * (qc + 1)]),
                    (v_r[:, p, 2 * c + h, :]),
                    start=(first and qc == 0),
                    stop=(stop and qc == 3),
                    skip_group_check=True,
                )

    def emit_drain(blk):
        p, qb = blk // QB, blk % QB
        for h in range(2):
            cs = work.tile([128, 260], F32, tag="cs", name="cs", bufs=3)
            nc.vector.tensor_scalar_add(cs[:], ctx_of[blk][h][:], 0.0)
            r0 = ((p * 2 + h) * QB + qb) * 128
            nc.sync.dma_start(out[r0 : r0 + 128, :], cs[:])
        del ctx_of[blk]
        del started[blk]

    # ---- preamble: warmup + the critical-path DMAs + pair-0 tb0 units ----
    # DMA serializes at ~2.9us/MB on the DMA fabric; the SP HWDGE queue is
    # in-order, so the queue order is exactly the need order: wk-p0, xk0,
    # wq-p0, xq0, cst unblock the first scores; the rest queues behind in
    # first-use order. The xv tb2/tb3 re-stages (which must WAIT on V-unit
    # consumers) go on the Pool engine's SWDGE queue so their waits cannot
    # head-block the SP queue.
    warmup(WARMUP_N)
    load_w_pair0("k")
    xk = [load_x_stage(xkT, "k", 0)]
    load_w_pair0("q")
    load_xq_res_tb(0)
    nc.sync.dma_start(cst_sb[:], cst[:])
    nc.sync.dma_start(bv_rep[:], bv[:])
    kq_unit(xk[0], "k", kt_sb, bk_sb, 0, 0)
    q_unit(0, 0)
    xk.append(load_x_stage(xkT, "k", 1))
    load_w("v")
    xv_t = {0: load_x_stage(xvT, "v", 0)}
    load_xq_res_tb(1)
    xk.append(load_x_stage(xkT, "k", 2))
    xv_t[1] = load_x_stage(xvT, "v", 1)
    xk.append(load_x_stage(xkT, "k", 3))
    load_xq_res_tb(2)
    load_w_rest("k")
    load_w_rest("q")
    load_xq_res_tb(3)

    # ---- filler schedule: slot -> [callables] ----
    filler = {}

    def add_filler(slot, fn):
        filler.setdefault(slot, []).append(fn)

    # dependency-free warmup batches bridge the early DMA-wait region
    # without blocking the in-order PE queue
    add_filler(1, lambda: warmup(5))
    add_filler(3, lambda: warmup(5))

    # K p0 tb1-3 as soon as their x lands
    add_filler(0, lambda: kq_unit(xk[1], "k", kt_sb, bk_sb, 0, 1))
    add_filler(4, lambda: kq_unit(xk[2], "k", kt_sb, bk_sb, 0, 2))
    add_filler(8, lambda: kq_unit(xk[3], "k", kt_sb, bk_sb, 0, 3))

    # V: one unit per slot for slots 5..20; xv tb2/3 re-staged via Pool DMA
    def stage_xv(tb):
        xt = xpool.tile([128, CCH * 512], BF16, tag="xv", name="xv", bufs=2)
        nc.gpsimd.dma_start(
            xt[:].rearrange("p (c t) -> p c t", c=CCH),
            xvT[:].rearrange("(c p) t -> p c t", p=128)[
                :, :, 512 * tb : 512 * (tb + 1)
            ],
        )
        xv_t[tb] = xt

    add_filler(11, lambda: stage_xv(2))
    add_filler(15, lambda: stage_xv(3))
    for tb in range(TB):
        for j in range(4):
            add_filler(7 + 4 * tb + j,
                       lambda tb=tb, j=j: v_unit(xv_t[tb], tb, j))
    # Q(p, qb) spread to just ahead of block (p,qb) at slot 64p+16qb
    for p in range(NPAIR):
        for qb in range(QB):
            if p == 0 and qb == 0:
                continue
            slot = 64 * p + 16 * qb - 7
            if p == 3:
                slot = 150 + 3 * qb   # keep blocks 10-15 filler-free
            add_filler(slot, lambda p=p, qb=qb: q_unit(p, qb))
    # later pairs: K units (x_k stays resident) ahead of slot 64*p
    for p in range(1, NPAIR):
        base = 64 * p if p < 3 else 156
        for tb in range(TB):
            add_filler(base - 20 + 4 * tb,
                       lambda p=p, tb=tb: kq_unit(xk[tb], "k", kt_sb,
                                                  bk_sb, p, tb))

    # ---- the flat stream ----
    pend = []     # (due_slot, blk, c, pt)
    stage2q = []  # (due_slot, ti, pt)
    slot = 0
    for blk in range(NBLK):
        p, qb = blk // QB, blk % QB
        q_sl = slice(512 * qb, 512 * (qb + 1))
        dve_cs = _dve_positions(blk)
        for c in range(TCH):
            kt_sl = slice(128 * c, 128 * (c + 1))
            use_mmp = c in dve_cs and blk >= 10
            if use_mmp:
                schalves = [
                    psum.tile([128, 512], F32, tag="mmp", bufs=2, name="ps")
                    for _ in range(2)
                ]
            else:
                sc = psum.tile([128, 1024], F32, tag="mm", bufs=2)
            for h in (0, 1):
                hp = slice(64 * h, 64 * (h + 1))
                nc.tensor.matmul(
                    schalves[h][:, 0:512] if use_mmp
                    else sc[:, 512 * h : 512 * (h + 1)],
                    (kt_sb[p][hp, kt_sl]),
                    (qt_sb[p][hp, q_sl]),
                    start=True,
                    stop=True,
                )
            if c in dve_cs:
                if use_mmp:
                    pt, ti = emit_exp_split1_mmp(schalves, c)
                else:
                    pt, ti = emit_exp_split1(sc, c)
                stage2q.append((slot + 2, ti, pt))
                lag = max(_blk_lag(blk), PV_LAG_DVE)
            else:
                pt = emit_exp_scalar(sc, c)
                lag = _blk_lag(blk)
            pend.append((slot + lag, blk, c, pt))
            pend.sort(key=lambda e: e[0])
            while stage2q and stage2q[0][0] <= slot:
                _, ti2, pt2 = stage2q.pop(0)
                emit_exp_split2(ti2, pt2)
            while pend and pend[0][0] <= slot:
                _, b2, c2, pt2 = pend.pop(0)
                done_emitting = b2 < blk or (b2 == blk and c == TCH - 1)
                stop = done_emitting and not any(e[1] == b2 for e in pend)
                emit_pv(b2, c2, pt2, stop)
                if stop:
                    emit_drain(b2)
            for fn in filler.pop(slot, []):
                fn()
            slot += 1
    while stage2q:
        _, ti2, pt2 = stage2q.pop(0)
        emit_exp_split2(ti2, pt2)
    while pend:
        _, b2, c2, pt2 = pend.pop(0)
        stop = not any(e[1] == b2 for e in pend)
        emit_pv(b2, c2, pt2, stop)
        if stop:
            emit_drain(b2)
    for sl in sorted(filler):
        for fn in filler.pop(sl):
            fn()

    pools.close()


def make_in_maps(x_q, x_k, x_v, att_mask, W_q, b_q, W_k, b_k, W_v, b_v):
    import ml_dtypes

    f = np.float32
    bf = ml_dtypes.bfloat16
    x_q, x_k, x_v = (np.asarray(a, f) for a in (x_q, x_k, x_v))
    att_mask = np.asarray(att_mask, f)
    W_q, W_k, W_v = (np.asarray(a, f) for a in (W_q, W_k, W_v))
    b_q, b_k, b_v = (np.asarray(a, f) for a in (b_q, b_k, b_v))

    in_maps = []
    for core in range(NCORES):
        b, g = divmod(core, TP)
        fsl = slice(DG * g, DG * (g + 1))
        mask_col = np.ascontiguousarray(
            att_mask[b, 0, 0].reshape(TCH, 128).T)           # [128, TCH]
        dve_b = (2.0 ** 23 * EXP_B0
                 + mask_col.astype(np.float64) * (2.0 ** 23 * LOG2E)
                 ).astype(np.float32)
        dve_a = np.full((128, 1), EXP_A, np.float32)
        m_and = np.full((128, 1), 0x007FFFFF, np.int32).view(np.float32)
        m_orr = np.full((128, 1), 0x3F800000, np.int32).view(np.float32)
        in_maps.append(
            {
                "xqT": np.ascontiguousarray(x_q[b].T.astype(bf)),
                "xkT": np.ascontiguousarray(x_k[b].T.astype(bf)),
                "xvT": np.ascontiguousarray(x_v[b].T.astype(bf)),
                "wqT": np.ascontiguousarray(W_q[fsl, :].T.astype(bf)),
                "wkT": np.ascontiguousarray(W_k[fsl, :].T.astype(bf)),
                "wvT": np.ascontiguousarray(W_v[fsl, :].T.astype(bf)),
                "cst": np.ascontiguousarray(
                    np.concatenate(
                        [
                            b_q[fsl].reshape(NPAIR, 128).T,
                            b_k[fsl].reshape(NPAIR, 128).T,
                            mask_col,
                            dve_b,
                            dve_a,
                            m_and,
                            m_orr,
                        ],
                        axis=1,
                    )
                ),
                "bv": np.ascontiguousarray(
                    np.broadcast_to(b_v[fsl].reshape(1, DG), (128, DG))
                ).astype(bf),
            }
        )
    return in_maps


def kernel(x_q, x_k, x_v, att_mask, W_q, b_q, W_k, b_k, W_v, b_v):
    global _CACHED
    if _CACHED is None:
        _CACHED = _build_core_program()
    nc = _CACHED

    in_maps = make_in_maps(
        x_q, x_k, x_v, att_mask, W_q, b_q, W_k, b_k, W_v, b_v
    )

    import os

    global LAST_RESULTS
    trace = TRACE or os.environ.get("BASS_KERNEL_TRACE", "") == "1"
    try:
        res = run_bass_kernel_spmd(nc, in_maps, list(range(NCORES)), trace=trace)
    except Exception:
        if not trace:
            raise
        res = run_bass_kernel_spmd(nc, in_maps, list(range(NCORES)))
    LAST_RESULTS = res

    full = np.empty((B, S, H), np.float32)
    for core in range(NCORES):
        b, g = divmod(core, TP)
        r = res.results[core]["out"].reshape(NPAIR, 2, QB, 128, 4, 65)
        ctx = r[..., 0:64] / r[..., 64:65]          # [p, h, qb, i, qc, d]
        full[b, :, DG * g : DG * (g + 1)] = (
            ctx.transpose(2, 4, 3, 0, 1, 5).reshape(S, DG)
        )
    return full


# revision 45
# speedup vs baseline: 1.1479x; 1.0364x over previous
"""Multi-head attention (B=4, S=2048, H=1024, NH=16) on 8 TRN2 NeuronCores.

Sharding: data-parallel over batch (4) x tensor-parallel over heads (2 groups
of 8 heads). Core c handles batch c//2, head-group c%2 (features 512*(c%2)..).
The host pre-transposes x to x^T [H, S] and W to W^T [H, DG] (bf16).

v2: flat software-pipelined attention stream. Per-core engine budgets are
PE ~247us (projections 82 + scores 109 + PV 55), ScalarE ~267us if it does
every exp, DVE ~44us. The v1 kernel lost ~60us to PE queue-head stalls
(PV(c) emitted right after scores(c) waits on chunk c's exp; the PE wait
queue is in-order) and ran ScalarE as the binding engine. This version:
  1. offloads the exp of 40/256 score chunks to the DVE via a 4-instruction
     bit-trick exp2 (int32-convert Schraudolph + mantissa-domain cubic
     correction, ~0.35% max elem error, HW-validated), leaving ScalarE
     ~224us and DVE ~234us, both under the PE floor;
  2. emits one flat stream over all 512 (pair, qblock, chunk) slots with PV
     lagged 2 slots behind scores (7 for DVE chunks, whose exp chain has
     ~4.6us latency), so the PE never parks on an unready PV; block
     boundaries overlap because the next block's scores are emitted before
     the previous block's PV flush;
  3. weaves projection units (8 accumulating matmuls + 1 DVE drain) into the
     stream as PE filler in dependency order: K/Q for pair 0 and all of V
     front-loaded, later pairs' K/Q (with their x re-stage DMAs) spread
     ahead of first use.
PSUM: scores 2x[128,1024] (4 banks) + proj 2x[128,512] (2) + ctx 2x[128,260]
(2) = 8 banks exactly.
"""

from contextlib import ExitStack

import numpy as np

import concourse.mybir as mybir
import concourse.tile as tile
from concourse import bacc
from concourse.alu_op_type import AluOpType
from concourse.bass_utils import run_bass_kernel_spmd

B, S, H, NH, HD = 4, 2048, 1024, 16, 64
NCORES = 8
DP, TP = 4, 2            # batch-parallel x head-group-parallel
HG = NH // TP            # 8 heads per core
DG = HG * HD             # 512 features per core
NPAIR = HG // 2          # 4 head pairs (128 features each)
CCH = H // 128           # 8 contraction chunks for projections
TB = S // 512            # 4 token blocks of 512
TCH = S // 128           # 16 token chunks of 128
QB = S // 512            # 4 q-blocks of 512
F32 = mybir.dt.float32
I32 = mybir.dt.int32
BF16 = mybir.dt.bfloat16

# --- DVE exp2 constants: exp(s/8) ~= y*(m^2 + s3*m + t3)*m/y-scale with
# y = bitcast_f32(int32(s*A + Bcol)), m = bitcast_f32((y&MANT)|ONE) --------
EXP_S3 = -2.9705181372354152
EXP_T3 = 6.234666527897044
EXP_K = 4.278858943597909
LOG2E = float(np.log2(np.e))
EXP_A = 2.0 ** 23 * LOG2E / 8.0                  # score -> w scale
EXP_B0 = 127.0 - float(np.log2(EXP_K))           # exponent bias
AND_IMM = float(0x007FFFFF)
OR_IMM = float(0x3F800000)

PV_LAG_SC = 2
PV_LAG_DVE = 6
WARMUP_N = 46
PT_BUFS = 16

_CACHED = None
LAST_RESULTS = None   # BassKernelResults of the most recent run (for test.py)
TRACE = False         # set True (or BASS_KERNEL_TRACE=1) to profile the run


def _register_exp2_op():
    """Register the 6-stage EXP2_FIXUP_ANT custom DVE op (idempotent).

    out = y * ((m + imm2) * m + in1),  m = bitcast_f32((bits(y) & s0) | s1)
    With y = bitcast_f32(int32(score*A + B)) this completes the Schraudolph
    exp2 with a mantissa-domain cubic correction (~0.35% max elem error).
    The op registers through dve_ops' own extension points, so the per-NEFF
    uop table, CoreSim reference, and ISA row all flow normally.
    """
    import concourse.dve_ops as dops
    from concourse.dve_spec import Spec, Src0, Src1, C0, C1, C2, Bin, AluOp, \
        lower
    from concourse.dve_uop import DveOpSpec

    if "EXP2_FIXUP_ANT" in dops._SUB_OPCODE_FOR_NAME:
        return next(o for o in dops.OPS if o.name == "EXP2_FIXUP_ANT")

    def _ref(in0, in1, s0, s1, imm2):
        t = np.asarray(in0, np.float32).view(np.int32)
        s0b = np.asarray(s0, np.float32).view(np.int32)
        s1b = np.asarray(s1, np.float32).view(np.int32)
        m = ((t & s0b) | s1b).view(np.float32)
        y = t.view(np.float32)
        return (y * ((m + np.float32(imm2)) * m
                     + np.asarray(in1, np.float32))).astype(np.float32)

    a = Bin(AluOp.BITWISE_AND, Src0, C0)
    m = Bin(AluOp.BITWISE_OR, a, C1)
    spec = Spec(body=Src0 * ((m + C2) * m + Src1), reference=_ref)
    row = max(dops._SUB_OPCODE_FOR_NAME.values()) + 1
    assert row < 0x20
    uops = lower(spec, ver="v3")
    sha = DveOpSpec(name="EXP2_FIXUP_ANT", opcode=row, uops=uops,
                    rd1_en=True).sha("v3")
    op = dops.DveOp("EXP2_FIXUP_ANT", spec, subdim=False,
                    uops_sha={"v3": sha})
    dops.OPS.append(op)
    dops.CUSTOM_DVE_SPECS[op.name] = spec
    dops._SUB_OPCODE_FOR_NAME[op.name] = row
    return op


def _dve_positions(blk):
    # Back-weighted DVE offload (49 chunks): the early region is PE-bound
    # (V/K/Q filler), so ScalarE alone keeps pace there; later regions have
    # little filler and the exp stream binds, so 4 chunks/block move to
    # the DVE.
    if blk < 3:
        return ()
    if blk == 3:
        return (9,)
    return (1, 5, 9, 13)


def _blk_lag(blk):
    # tapered PV lag: deep early (V projections land ~25us in), 2 at steady
    return max(PV_LAG_SC, 12 - blk)


def _build_core_program(repeat=1):
    nc = bacc.Bacc(
        "TRN2", target_bir_lowering=False, debug=False, enable_asserts=False
    )

    xqT = nc.declare_dram_parameter("xqT", [H, S], BF16, isOutput=False)
    xkT = nc.declare_dram_parameter("xkT", [H, S], BF16, isOutput=False)
    xvT = nc.declare_dram_parameter("xvT", [H, S], BF16, isOutput=False)
    wqT = nc.declare_dram_parameter("wqT", [H, DG], BF16, isOutput=False)
    wkT = nc.declare_dram_parameter("wkT", [H, DG], BF16, isOutput=False)
    wvT = nc.declare_dram_parameter("wvT", [H, DG], BF16, isOutput=False)
    # bq | bk | mask | dve_b | dve_a | and-mask | or-mask packed as
    # [128, 2*NPAIR+2*TCH+3] f32
    cst = nc.declare_dram_parameter("cst", [128, 2 * NPAIR + 2 * TCH + 3],
                                    F32, isOutput=False)
    bv = nc.declare_dram_parameter("bv", [128, DG], BF16, isOutput=False)
    # rows: ((pair*2 + head)*QB + qb)*128 + q_local; cols: qc*65 + (d | sumexp)
    out = nc.declare_dram_parameter("out", [NPAIR * 2 * QB * 128, 260], F32,
                                    isOutput=True)

    with tile.TileContext(nc) as tc:
        for _ in range(repeat):
            _emit(tc, nc, xqT, xkT, xvT, wqT, wkT, wvT, cst, bv, out)

    nc.compile()
    return nc


def _emit(tc, nc, xqT, xkT, xvT, wqT, wkT, wvT, cst, bv, out):
    Exp = mybir.ActivationFunctionType.Exp

    pools = ExitStack()
    const = pools.enter_context(tc.tile_pool(name="const", bufs=1))
    persist = pools.enter_context(tc.tile_pool(name="persist", bufs=1))
    xpool = pools.enter_context(tc.tile_pool(name="xpool", bufs=2))
    work = pools.enter_context(tc.tile_pool(name="work", bufs=6))
    psum = pools.enter_context(tc.tile_pool(name="psum", bufs=1, space="PSUM"))

    # ---- constants / weights ----
    cst_sb = const.tile([128, 2 * NPAIR + 2 * TCH + 3], F32, tag="cst")
    bq_sb = cst_sb[:, 0:NPAIR]
    bk_sb = cst_sb[:, NPAIR : 2 * NPAIR]
    mask_sb = cst_sb[:, 2 * NPAIR : 2 * NPAIR + TCH]
    dveb_sb = cst_sb[:, 2 * NPAIR + TCH : 2 * NPAIR + 2 * TCH]
    dvea_sb = cst_sb[:, 2 * NPAIR + 2 * TCH : 2 * NPAIR + 2 * TCH + 1]
    mand_sb = cst_sb[:, 2 * NPAIR + 2 * TCH + 1 : 2 * NPAIR + 2 * TCH + 2]
    morr_sb = cst_sb[:, 2 * NPAIR + 2 * TCH + 2 : 2 * NPAIR + 2 * TCH + 3]
    bv_rep = const.tile([128, DG], BF16, tag="bv_rep")
    t3c_sb = const.tile([128, 1024], F32, tag="t3c")
    nc.gpsimd.memset(t3c_sb[:], float(EXP_T3))
    exp2_op = _register_exp2_op()

    # PE p-state warm-up through the initial DMA era
    def warmup(n):
        wz = xpool.tile([128, 256], BF16, tag="warm", name="wz", bufs=1)
        nc.gpsimd.memset(wz[:], 0.0)
        for i in range(n):
            ps = psum.tile([128, 512], F32, tag="mmp", bufs=2, name="ps")
            nc.tensor.matmul(ps[:, 0:256], wz[:, 0:128], wz[:], start=True,
                             stop=True)

    # weights as [128, cch*DG]; pair-0 slice loads first
    w_sb = {}
    w_dram = {"k": wkT, "q": wqT, "v": wvT}

    def _w_views(name):
        if name not in w_sb:
            w_sb[name] = const.tile(
                [128, CCH * DG], BF16, tag=f"w{name}", name=f"w{name}"
            )
        w = w_sb[name]
        wv4 = w[:].rearrange("p (c g d) -> p c g d", c=CCH, g=NPAIR)
        sv4 = w_dram[name][:].rearrange("(c p) (g d) -> p c g d", p=128,
                                        g=NPAIR)
        return wv4, sv4

    def load_w_pair0(name):
        # pairs 0-1 as one 512B-run slice: same DMA time as a 256B-run
        # pair-0-only load (which pays the <512B latency multiplier)
        _w_views(name)
        nc.sync.dma_start(
            w_sb[name][:].rearrange("p (c d) -> p c d", c=CCH)[:, :, 0:256],
            w_dram[name][:].rearrange("(c p) d -> p c d", p=128)[:, :, 0:256],
        )

    def load_w_rest(name):
        nc.sync.dma_start(
            w_sb[name][:].rearrange("p (c d) -> p c d", c=CCH)[:, :, 256:],
            w_dram[name][:].rearrange("(c p) d -> p c d", p=128)[:, :, 256:],
        )

    def load_w(name):
        _w_views(name)
        nc.sync.dma_start(
            w_sb[name][:].rearrange("p (c d) -> p c d", c=CCH),
            w_dram[name][:].rearrange("(c p) d -> p c d", p=128),
        )

    # ---- persistent activations ----
    qt_sb = [
        persist.tile([128, S], BF16, tag=f"qt{p}", name=f"qt{p}")
        for p in range(NPAIR)
    ]
    kt_sb = [
        persist.tile([128, S], BF16, tag=f"kt{p}", name=f"kt{p}")
        for p in range(NPAIR)
    ]
    v_sb = persist.tile([128, NPAIR * TCH * 2 * 65], BF16, tag="v")
    v_r = v_sb[:].rearrange("t (p c2 d) -> t p c2 d", p=NPAIR, c2=TCH * 2)
    nc.gpsimd.memset(v_r[:, :, :, 64:65], 1.0)

    # resident x_q^T [128, cch*S] bf16 (32KB/partition)
    xq_res = persist.tile([128, CCH * S], BF16, tag="xq_res")

    def load_xq_res_tb(tb):
        t_sl = slice(512 * tb, 512 * (tb + 1))
        nc.sync.dma_start(
            xq_res[:].rearrange("p (c t) -> p c t", c=CCH)[:, :, t_sl],
            xqT[:].rearrange("(c p) t -> p c t", p=128)[:, :, t_sl],
        )

    def load_x_stage(xT, key, tb):
        xt = xpool.tile(
            [128, CCH * 512], BF16, tag=f"x{key}", name=f"x{key}",
            bufs=4 if key == "k" else 2,
        )
        nc.sync.dma_start(
            xt[:].rearrange("p (c t) -> p c t", c=CCH),
            xT[:].rearrange("(c p) t -> p c t", p=128)[
                :, :, 512 * tb : 512 * (tb + 1)
            ],
        )
        return xt

    # ---- projection units (8 matmuls + 1 DVE drain each) ----
    def kq_unit(xt, wkey, dst, bias_sb, p, tb):
        ps = psum.tile([128, 512], F32, tag="mmp", bufs=2, name="ps")
        for c in range(CCH):
            nc.tensor.matmul(
                ps[:, 0:512],
                (w_sb[wkey][:, c * DG + 128 * p : c * DG + 128 * (p + 1)]),
                (xt[:, 512 * c : 512 * (c + 1)]),
                start=(c == 0),
                stop=(c == CCH - 1),
            )
        nc.vector.tensor_scalar_add(
            dst[p][:, 512 * tb : 512 * (tb + 1)],
            ps[:, 0:512],
            bias_sb[:, p : p + 1],
        )

    def q_unit(p, tb):
        ps = psum.tile([128, 512], F32, tag="mmp", bufs=2, name="ps")
        for c in range(CCH):
            nc.tensor.matmul(
                ps[:, 0:512],
                (w_sb["q"][:, c * DG + 128 * p : c * DG + 128 * (p + 1)]),
                (xq_res[:, c * S + 512 * tb : c * S + 512 * (tb + 1)]),
                start=(c == 0),
                stop=(c == CCH - 1),
            )
        nc.vector.tensor_scalar_add(
            qt_sb[p][:, 512 * tb : 512 * (tb + 1)],
            ps[:, 0:512],
            bq_sb[:, p : p + 1],
        )

    def v_unit(xt, tb, j):
        c = 4 * tb + j
        ps = psum.tile([128, 512], F32, tag="mmp", bufs=2, name="ps")
        for cc in range(CCH):
            nc.tensor.matmul(
                ps[:, 0:512],
                (xt[:, 512 * cc + 128 * j : 512 * cc + 128 * (j + 1)]),
                (w_sb["v"][:, cc * DG : (cc + 1) * DG]),
                start=(cc == 0),
                stop=(cc == CCH - 1),
            )
        nc.vector.tensor_add(
            v_r[:, :, 2 * c : 2 * c + 2, 0:64],
            ps[:, 0:512].rearrange("t (p h d) -> t p h d", p=NPAIR, h=2),
            bv_rep[:].rearrange("t (p h d) -> t p h d", p=NPAIR, h=2),
        )

    # ---- exp paths ----
    def emit_exp_scalar(sc, c):
        pt = work.tile([128, 1024], BF16, tag="pt", bufs=PT_BUFS)
        nc.scalar.activation(
            pt[:], sc[:], Exp, bias=mask_sb[:, c : c + 1], scale=0.125
        )
        return pt

    # Split-chunk exp: ScalarE takes head 0, the DVE takes head 1 via the
    # pipelined int32-convert (frees its sc half fast) + custom fixup two
    # slots later.
    def emit_exp_split1(sc, c):
        pt = work.tile([128, 1024], BF16, tag="pt", bufs=PT_BUFS)
        ti = work.tile([128, 1024], I32, tag="ti", bufs=2)
        nc.vector.tensor_scalar(ti[:], sc[:], dvea_sb[:, 0:1],
                                dveb_sb[:, c : c + 1],
                                AluOpType.mult, AluOpType.add)
        return pt, ti

    def emit_exp_split1_mmp(schalves, c):
        pt = work.tile([128, 1024], BF16, tag="pt", bufs=PT_BUFS)
        ti = work.tile([128, 1024], I32, tag="ti", bufs=2)
        for h in (0, 1):
            hs = slice(512 * h, 512 * (h + 1))
            nc.vector.tensor_scalar(ti[:, hs], schalves[h][:, 0:512],
                                    dvea_sb[:, 0:1], dveb_sb[:, c : c + 1],
                                    AluOpType.mult, AluOpType.add)
        return pt, ti

    def emit_exp_split2(ti, pt):
        nc.vector._custom_dve(exp2_op, out=pt[:],
                              in0=ti[:].bitcast(F32),
                              in1=t3c_sb[:], s0=mand_sb, s1=morr_sb,
                              imm2=float(EXP_S3))

    # ---- flat attention pipeline over slots ----
    NBLK = NPAIR * QB

    ctx_of = {}
    started = {}

    def emit_pv(blk, c, pt, stop):
        p, qb = blk // QB, blk % QB
        if blk not in ctx_of:
            ctx_of[blk] = [
                psum.tile([128, 260], F32, tag=f"ctx{h}", name=f"ctx{h}")
                for h in range(2)
            ]
            started[blk] = [False, False]
        for h in range(2):
            first = not started[blk][h]
            started[blk][h] = True
            for qc in range(4):
                nc.tensor.matmul(
                    ctx_of[blk][h][:, 65 * qc : 65 * qc + 65],
                    (pt[:, 512 * h + 128 * qc : 512 * h + 128 * (qc + 1)]),
                    (v_r[:, p, 2 * c + h, :]),
                    start=(first and qc == 0),
                    stop=(stop and qc == 3),
                    skip_group_check=True,
                )

    def emit_drain(blk):
        p, qb = blk // QB, blk % QB
        for h in range(2):
            cs = work.tile([128, 260], F32, tag="cs", name="cs", bufs=3)
            nc.vector.tensor_scalar_add(cs[:], ctx_of[blk][h][:], 0.0)
            r0 = ((p * 2 + h) * QB + qb) * 128
            nc.sync.dma_start(out[r0 : r0 + 128, :], cs[:])
        del ctx_of[blk]
        del started[blk]

    # ---- preamble: warmup + the critical-path DMAs + pair-0 tb0 units ----
    # DMA serializes at ~2.9us/MB on the DMA fabric; the SP HWDGE queue is
    # in-order, so the queue order is exactly the need order: wk-p0, xk0,
    # wq-p0, xq0, cst unblock the first scores; the rest queues behind in
    # first-use order. The xv tb2/tb3 re-stages (which must WAIT on V-unit
    # consumers) go on the Pool engine's SWDGE queue so their waits cannot
    # head-block the SP queue.
    warmup(WARMUP_N)
    load_w_pair0("k")
    xk = [load_x_stage(xkT, "k", 0)]
    load_w_pair0("q")
    load_xq_res_tb(0)
    nc.sync.dma_start(cst_sb[:], cst[:])
    nc.sync.dma_start(bv_rep[:], bv[:])
    kq_unit(xk[0], "k", kt_sb, bk_sb, 0, 0)
    q_unit(0, 0)
    xk.append(load_x_stage(xkT, "k", 1))
    xk.append(load_x_stage(xkT, "k", 2))
    load_w("v")
    xv_t = {0: load_x_stage(xvT, "v", 0)}
    load_xq_res_tb(1)
    xk.append(load_x_stage(xkT, "k", 3))
    xv_t[1] = load_x_stage(xvT, "v", 1)
    load_xq_res_tb(2)
    load_w_rest("k")
    load_w_rest("q")
    load_xq_res_tb(3)

    # ---- filler schedule: slot -> [callables] ----
    filler = {}

    def add_filler(slot, fn):
        filler.setdefault(slot, []).append(fn)

    # dependency-free warmup batches bridge the early DMA-wait region
    # without blocking the in-order PE queue
    add_filler(1, lambda: warmup(5))
    add_filler(3, lambda: warmup(5))

    # K p0 tb1-3 as soon as their x lands
    add_filler(0, lambda: kq_unit(xk[1], "k", kt_sb, bk_sb, 0, 1))
    add_filler(4, lambda: kq_unit(xk[2], "k", kt_sb, bk_sb, 0, 2))
    add_filler(8, lambda: kq_unit(xk[3], "k", kt_sb, bk_sb, 0, 3))

    # V: one unit per slot for slots 5..20; xv tb2/3 re-staged via Pool DMA
    def stage_xv(tb):
        xt = xpool.tile([128, CCH * 512], BF16, tag="xv", name="xv", bufs=2)
        nc.gpsimd.dma_start(
            xt[:].rearrange("p (c t) -> p c t", c=CCH),
            xvT[:].rearrange("(c p) t -> p c t", p=128)[
                :, :, 512 * tb : 512 * (tb + 1)
            ],
        )
        xv_t[tb] = xt

    add_filler(11, lambda: stage_xv(2))
    add_filler(15, lambda: stage_xv(3))
    for tb in range(TB):
        for j in range(4):
            add_filler(7 + 4 * tb + j,
                       lambda tb=tb, j=j: v_unit(xv_t[tb], tb, j))
    # Q(p, qb) spread to just ahead of block (p,qb) at slot 64p+16qb
    for p in range(NPAIR):
        for qb in range(QB):
            if p == 0 and qb == 0:
                continue
            slot = 64 * p + 16 * qb - 7
            if p == 3:
                slot = 150 + 3 * qb   # keep blocks 10-15 filler-free
            add_filler(slot, lambda p=p, qb=qb: q_unit(p, qb))
    # later pairs: K units (x_k stays resident) ahead of slot 64*p
    for p in range(1, NPAIR):
        base = 64 * p if p < 3 else 156
        for tb in range(TB):
            add_filler(base - 20 + 4 * tb,
                       lambda p=p, tb=tb: kq_unit(xk[tb], "k", kt_sb,
                                                  bk_sb, p, tb))

    # ---- the flat stream ----
    pend = []     # (due_slot, blk, c, pt)
    stage2q = []  # (due_slot, ti, pt)
    slot = 0
    for blk in range(NBLK):
        p, qb = blk // QB, blk % QB
        q_sl = slice(512 * qb, 512 * (qb + 1))
        dve_cs = _dve_positions(blk)
        for c in range(TCH):
            kt_sl = slice(128 * c, 128 * (c + 1))
            use_mmp = c in dve_cs and blk >= 10
            if use_mmp:
                schalves = [
                    psum.tile([128, 512], F32, tag="mmp", bufs=2, name="ps")
                    for _ in range(2)
                ]
            else:
                sc = psum.tile([128, 1024], F32, tag="mm", bufs=2)
            for h in (0, 1):
                hp = slice(64 * h, 64 * (h + 1))
                nc.tensor.matmul(
                    schalves[h][:, 0:512] if use_mmp
                    else sc[:, 512 * h : 512 * (h + 1)],
                    (kt_sb[p][hp, kt_sl]),
                    (qt_sb[p][hp, q_sl]),
                    start=True,
                    stop=True,
                )
            if c in dve_cs:
                if use_mmp:
                    pt, ti = emit_exp_split1_mmp(schalves, c)
                else:
                    pt, ti = emit_exp_split1(sc, c)
                stage2q.append((slot + 2, ti, pt))
                lag = max(_blk_lag(blk), PV_LAG_DVE)
            else:
                pt = emit_exp_scalar(sc, c)
                lag = _blk_lag(blk)
            pend.append((slot + lag, blk, c, pt))
            pend.sort(key=lambda e: e[0])
            while stage2q and stage2q[0][0] <= slot:
                _, ti2, pt2 = stage2q.pop(0)
                emit_exp_split2(ti2, pt2)
            while pend and pend[0][0] <= slot:
                _, b2, c2, pt2 = pend.pop(0)
                done_emitting = b2 < blk or (b2 == blk and c == TCH - 1)
                stop = done_emitting and not any(e[1] == b2 for e in pend)
                emit_pv(b2, c2, pt2, stop)
                if stop:
                    emit_drain(b2)
            for fn in filler.pop(slot, []):
                fn()
            slot += 1
    while stage2q:
        _, ti2, pt2 = stage2q.pop(0)
        emit_exp_split2(ti2, pt2)
    while pend:
        _, b2, c2, pt2 = pend.pop(0)
        stop = not any(e[1] == b2 for e in pend)
        emit_pv(b2, c2, pt2, stop)
        if stop:
            emit_drain(b2)
    for sl in sorted(filler):
        for fn in filler.pop(sl):
            fn()

    pools.close()


def make_in_maps(x_q, x_k, x_v, att_mask, W_q, b_q, W_k, b_k, W_v, b_v):
    import ml_dtypes

    f = np.float32
    bf = ml_dtypes.bfloat16
    x_q, x_k, x_v = (np.asarray(a, f) for a in (x_q, x_k, x_v))
    att_mask = np.asarray(att_mask, f)
    W_q, W_k, W_v = (np.asarray(a, f) for a in (W_q, W_k, W_v))
    b_q, b_k, b_v = (np.asarray(a, f) for a in (b_q, b_k, b_v))

    in_maps = []
    for core in range(NCORES):
        b, g = divmod(core, TP)
        fsl = slice(DG * g, DG * (g + 1))
        mask_col = np.ascontiguousarray(
            att_mask[b, 0, 0].reshape(TCH, 128).T)           # [128, TCH]
        dve_b = (2.0 ** 23 * EXP_B0
                 + mask_col.astype(np.float64) * (2.0 ** 23 * LOG2E)
                 ).astype(np.float32)
        dve_a = np.full((128, 1), EXP_A, np.float32)
        m_and = np.full((128, 1), 0x007FFFFF, np.int32).view(np.float32)
        m_orr = np.full((128, 1), 0x3F800000, np.int32).view(np.float32)
        in_maps.append(
            {
                "xqT": np.ascontiguousarray(x_q[b].T.astype(bf)),
                "xkT": np.ascontiguousarray(x_k[b].T.astype(bf)),
                "xvT": np.ascontiguousarray(x_v[b].T.astype(bf)),
                "wqT": np.ascontiguousarray(W_q[fsl, :].T.astype(bf)),
                "wkT": np.ascontiguousarray(W_k[fsl, :].T.astype(bf)),
                "wvT": np.ascontiguousarray(W_v[fsl, :].T.astype(bf)),
                "cst": np.ascontiguousarray(
                    np.concatenate(
                        [
                            b_q[fsl].reshape(NPAIR, 128).T,
                            b_k[fsl].reshape(NPAIR, 128).T,
                            mask_col,
                            dve_b,
                            dve_a,
                            m_and,
                            m_orr,
                        ],
                        axis=1,
                    )
                ),
                "bv": np.ascontiguousarray(
                    np.broadcast_to(b_v[fsl].reshape(1, DG), (128, DG))
                ).astype(bf),
            }
        )
    return in_maps


def kernel(x_q, x_k, x_v, att_mask, W_q, b_q, W_k, b_k, W_v, b_v):
    global _CACHED
    if _CACHED is None:
        _CACHED = _build_core_program()
    nc = _CACHED

    in_maps = make_in_maps(
        x_q, x_k, x_v, att_mask, W_q, b_q, W_k, b_k, W_v, b_v
    )

    import os

    global LAST_RESULTS
    trace = TRACE or os.environ.get("BASS_KERNEL_TRACE", "") == "1"
    try:
        res = run_bass_kernel_spmd(nc, in_maps, list(range(NCORES)), trace=trace)
    except Exception:
        if not trace:
            raise
        res = run_bass_kernel_spmd(nc, in_maps, list(range(NCORES)))
    LAST_RESULTS = res

    full = np.empty((B, S, H), np.float32)
    for core in range(NCORES):
        b, g = divmod(core, TP)
        r = res.results[core]["out"].reshape(NPAIR, 2, QB, 128, 4, 65)
        ctx = r[..., 0:64] / r[..., 64:65]          # [p, h, qb, i, qc, d]
        full[b, :, DG * g : DG * (g + 1)] = (
            ctx.transpose(2, 4, 3, 0, 1, 5).reshape(S, DG)
        )
    return full


# revision 58
# speedup vs baseline: 1.1588x; 1.0095x over previous
"""Multi-head attention (B=4, S=2048, H=1024, NH=16) on 8 TRN2 NeuronCores.

Sharding: data-parallel over batch (4) x tensor-parallel over heads (2 groups
of 8 heads). Core c handles batch c//2, head-group c%2 (features 512*(c%2)..).
The host pre-transposes x to x^T [H, S] and W to W^T [H, DG] (bf16).

v2 (281us vs 326us baseline): flat software-pipelined attention stream.
Per-core engine budgets: PE ~252us busy (projections 82 + scores 109 +
PV 56 + warmup), ScalarE ~267us if it did every exp, DVE idle. The v1
kernel lost ~60us to PE queue-head stalls (PV(c) emitted right after
scores(c) waits on chunk c's exp; the PE wait queue is in-order) and ran
ScalarE as the binding engine. This version:
  1. offloads the exp of 49/256 score chunks to the DVE via a 2-instruction
     bit-trick exp2: int32-convert Schraudolph (tensor_scalar mult/add with
     int32 convert-on-write, mask folded into the per-partition add) plus a
     runtime-registered 6-stage custom DVE op EXP2_FIXUP_ANT that applies a
     mantissa-domain cubic correction (out = y*((m+s3)*m*... +t3), ~0.35%
     max element error, HW-validated; softmax-level effect ~2e-4). The
     offload is back-weighted: the early region is PE-bound on projection
     filler, so ScalarE alone paces it; late blocks have no filler and the
     exp stream binds, so 4 chunks/block go to the DVE there.
  2. emits one flat stream over all 256 (pair, qblock, chunk) slots with PV
     lagged 2 slots behind scores (6 for DVE chunks; tapered to ~14 in the
     first blocks while the V projections land), so the in-order PE queue
     never parks on an unready PV; block boundaries overlap because the
     next block's scores are emitted before the previous block's PV flush.
     The DVE exp is itself pipelined: the int32 convert (which frees the
     score PSUM bank) is emitted at the chunk's slot, the fixup two slots
     later, so the 2-deep score ring never throttles on a busy DVE.
  3. weaves projection units (8 accumulating matmuls + 1 DVE drain) into
     the stream as PE filler in dependency order (K/Q pair 0 + all of V
     early, later pairs spread ahead of first use), and orders the startup
     DMA queue by first use (the DMA fabric moves ~2.9us/MB serially and is
     the startup constraint; weight slices load as >=512B runs to avoid the
     sub-512B 2x latency multiplier).
  4. for blocks >= 10 (filler-free), DVE-chunk scores accumulate in the
     then-idle projection PSUM banks instead of the shared score ring, so
     the ScalarE activation chain runs back-to-back at its 1038ns floor
     there (the late region is activation-bound).
PSUM: scores 2x[128,1024] (4 banks) + proj/dve-scores 2x[128,512] (2) +
ctx 2x[128,260] (2) = 8 banks exactly.
"""

from contextlib import ExitStack

import numpy as np

import concourse.mybir as mybir
import concourse.tile as tile
from concourse import bacc
from concourse.alu_op_type import AluOpType
from concourse.bass_utils import run_bass_kernel_spmd

B, S, H, NH, HD = 4, 2048, 1024, 16, 64
NCORES = 8
DP, TP = 4, 2            # batch-parallel x head-group-parallel
HG = NH // TP            # 8 heads per core
DG = HG * HD             # 512 features per core
NPAIR = HG // 2          # 4 head pairs (128 features each)
CCH = H // 128           # 8 contraction chunks for projections
TB = S // 512            # 4 token blocks of 512
TCH = S // 128           # 16 token chunks of 128
QB = S // 512            # 4 q-blocks of 512
F32 = mybir.dt.float32
I32 = mybir.dt.int32
BF16 = mybir.dt.bfloat16

# --- DVE exp2 constants: exp(s/8) ~= y*(m^2 + s3*m + t3)*m/y-scale with
# y = bitcast_f32(int32(s*A + Bcol)), m = bitcast_f32((y&MANT)|ONE) --------
EXP_S3 = -2.9705181372354152
EXP_T3 = 6.234666527897044
EXP_K = 4.278858943597909
LOG2E = float(np.log2(np.e))
EXP_A = 2.0 ** 23 * LOG2E / 8.0                  # score -> w scale
EXP_B0 = 127.0 - float(np.log2(EXP_K))           # exponent bias
PV_LAG_SC = 2
PV_LAG_DVE = 6
WARMUP_N = 46
PT_BUFS = 16

_CACHED = None
LAST_RESULTS = None   # BassKernelResults of the most recent run (for test.py)
TRACE = False         # set True (or BASS_KERNEL_TRACE=1) to profile the run


def _register_exp2_op():
    """Register the 6-stage EXP2_FIXUP_ANT custom DVE op (idempotent).

    out = y * ((m + imm2) * m + in1),  m = bitcast_f32((bits(y) & s0) | s1)
    With y = bitcast_f32(int32(score*A + B)) this completes the Schraudolph
    exp2 with a mantissa-domain cubic correction (~0.35% max elem error).
    The op registers through dve_ops' own extension points, so the per-NEFF
    uop table, CoreSim reference, and ISA row all flow normally.
    """
    import concourse.dve_ops as dops
    from concourse.dve_spec import Spec, Src0, Src1, C0, C1, C2, Bin, AluOp, \
        lower
    from concourse.dve_uop import DveOpSpec

    if "EXP2_FIXUP_ANT" in dops._SUB_OPCODE_FOR_NAME:
        return next(o for o in dops.OPS if o.name == "EXP2_FIXUP_ANT")

    def _ref(in0, in1, s0, s1, imm2):
        t = np.asarray(in0, np.float32).view(np.int32)
        s0b = np.asarray(s0, np.float32).view(np.int32)
        s1b = np.asarray(s1, np.float32).view(np.int32)
        m = ((t & s0b) | s1b).view(np.float32)
        y = t.view(np.float32)
        return (y * ((m + np.float32(imm2)) * m
                     + np.asarray(in1, np.float32))).astype(np.float32)

    a = Bin(AluOp.BITWISE_AND, Src0, C0)
    m = Bin(AluOp.BITWISE_OR, a, C1)
    spec = Spec(body=Src0 * ((m + C2) * m + Src1), reference=_ref)
    row = max(dops._SUB_OPCODE_FOR_NAME.values()) + 1
    assert row < 0x20
    uops = lower(spec, ver="v3")
    sha = DveOpSpec(name="EXP2_FIXUP_ANT", opcode=row, uops=uops,
                    rd1_en=True).sha("v3")
    op = dops.DveOp("EXP2_FIXUP_ANT", spec, subdim=False,
                    uops_sha={"v3": sha})
    dops.OPS.append(op)
    dops.CUSTOM_DVE_SPECS[op.name] = spec
    dops._SUB_OPCODE_FOR_NAME[op.name] = row
    return op


def _dve_positions(blk):
    # Back-weighted DVE offload (49 chunks): the early region is PE-bound
    # (V/K/Q filler), so ScalarE alone keeps pace there; later regions have
    # little filler and the exp stream binds, so 4 chunks/block move to
    # the DVE.
    if blk < 3:
        return ()
    if blk == 3:
        return (9,)
    return (1, 5, 9, 13)


def _blk_lag(blk):
    # tapered PV lag: deep early (V projections land ~25us in), 2 at steady
    return max(PV_LAG_SC, 14 - blk)


def _build_core_program(repeat=1):
    nc = bacc.Bacc(
        "TRN2", target_bir_lowering=False, debug=False, enable_asserts=False
    )

    xqT = nc.declare_dram_parameter("xqT", [H, S], BF16, isOutput=False)
    xkT = nc.declare_dram_parameter("xkT", [H, S], BF16, isOutput=False)
    xvT = nc.declare_dram_parameter("xvT", [H, S], BF16, isOutput=False)
    wqT = nc.declare_dram_parameter("wqT", [H, DG], BF16, isOutput=False)
    wkT = nc.declare_dram_parameter("wkT", [H, DG], BF16, isOutput=False)
    wvT = nc.declare_dram_parameter("wvT", [H, DG], BF16, isOutput=False)
    # bq | bk | mask | dve_b | dve_a | and-mask | or-mask packed as
    # [128, 2*NPAIR+2*TCH+3] f32
    cst = nc.declare_dram_parameter("cst", [128, 2 * NPAIR + 2 * TCH + 3],
                                    F32, isOutput=False)
    bv = nc.declare_dram_parameter("bv", [128, DG], BF16, isOutput=False)
    # rows: ((pair*2 + head)*QB + qb)*128 + q_local; cols: qc*65 + (d | sumexp)
    out = nc.declare_dram_parameter("out", [NPAIR * 2 * QB * 128, 260], F32,
                                    isOutput=True)

    with tile.TileContext(nc) as tc:
        for _ in range(repeat):
            _emit(tc, nc, xqT, xkT, xvT, wqT, wkT, wvT, cst, bv, out)

    nc.compile()
    return nc


def _emit(tc, nc, xqT, xkT, xvT, wqT, wkT, wvT, cst, bv, out):
    Exp = mybir.ActivationFunctionType.Exp

    pools = ExitStack()
    const = pools.enter_context(tc.tile_pool(name="const", bufs=1))
    persist = pools.enter_context(tc.tile_pool(name="persist", bufs=1))
    xpool = pools.enter_context(tc.tile_pool(name="xpool", bufs=2))
    work = pools.enter_context(tc.tile_pool(name="work", bufs=6))
    psum = pools.enter_context(tc.tile_pool(name="psum", bufs=1, space="PSUM"))

    # ---- constants / weights ----
    cst_sb = const.tile([128, 2 * NPAIR + 2 * TCH + 3], F32, tag="cst")
    bq_sb = cst_sb[:, 0:NPAIR]
    bk_sb = cst_sb[:, NPAIR : 2 * NPAIR]
    mask_sb = cst_sb[:, 2 * NPAIR : 2 * NPAIR + TCH]
    dveb_sb = cst_sb[:, 2 * NPAIR + TCH : 2 * NPAIR + 2 * TCH]
    dvea_sb = cst_sb[:, 2 * NPAIR + 2 * TCH : 2 * NPAIR + 2 * TCH + 1]
    mand_sb = cst_sb[:, 2 * NPAIR + 2 * TCH + 1 : 2 * NPAIR + 2 * TCH + 2]
    morr_sb = cst_sb[:, 2 * NPAIR + 2 * TCH + 2 : 2 * NPAIR + 2 * TCH + 3]
    bv_rep = const.tile([128, DG], BF16, tag="bv_rep")
    t3c_sb = const.tile([128, 1024], F32, tag="t3c")
    nc.gpsimd.memset(t3c_sb[:], float(EXP_T3))
    exp2_op = _register_exp2_op()

    # PE p-state warm-up through the initial DMA era
    def warmup(n):
        wz = xpool.tile([128, 256], BF16, tag="warm", name="wz", bufs=1)
        nc.gpsimd.memset(wz[:], 0.0)
        for i in range(n):
            ps = psum.tile([128, 512], F32, tag="mmp", bufs=2, name="ps")
            nc.tensor.matmul(ps[:, 0:256], wz[:, 0:128], wz[:], start=True,
                             stop=True)

    # weights as [128, cch*DG]; pair-0 slice loads first
    w_sb = {}
    w_dram = {"k": wkT, "q": wqT, "v": wvT}

    def _w_views(name):
        if name not in w_sb:
            w_sb[name] = const.tile(
                [128, CCH * DG], BF16, tag=f"w{name}", name=f"w{name}"
            )
        w = w_sb[name]
        wv4 = w[:].rearrange("p (c g d) -> p c g d", c=CCH, g=NPAIR)
        sv4 = w_dram[name][:].rearrange("(c p) (g d) -> p c g d", p=128,
                                        g=NPAIR)
        return wv4, sv4

    def load_w_pair0(name):
        # pairs 0-1 as one 512B-run slice: same DMA time as a 256B-run
        # pair-0-only load (which pays the <512B latency multiplier)
        _w_views(name)
        nc.sync.dma_start(
            w_sb[name][:].rearrange("p (c d) -> p c d", c=CCH)[:, :, 0:256],
            w_dram[name][:].rearrange("(c p) d -> p c d", p=128)[:, :, 0:256],
        )

    def load_w_rest(name):
        nc.sync.dma_start(
            w_sb[name][:].rearrange("p (c d) -> p c d", c=CCH)[:, :, 256:],
            w_dram[name][:].rearrange("(c p) d -> p c d", p=128)[:, :, 256:],
        )

    def load_w(name):
        _w_views(name)
        nc.sync.dma_start(
            w_sb[name][:].rearrange("p (c d) -> p c d", c=CCH),
            w_dram[name][:].rearrange("(c p) d -> p c d", p=128),
        )

    # ---- persistent activations ----
    qt_sb = [
        persist.tile([128, S], BF16, tag=f"qt{p}", name=f"qt{p}")
        for p in range(NPAIR)
    ]
    kt_sb = [
        persist.tile([128, S], BF16, tag=f"kt{p}", name=f"kt{p}")
        for p in range(NPAIR)
    ]
    v_sb = persist.tile([128, NPAIR * TCH * 2 * 65], BF16, tag="v")
    v_r = v_sb[:].rearrange("t (p c2 d) -> t p c2 d", p=NPAIR, c2=TCH * 2)
    nc.gpsimd.memset(v_r[:, :, :, 64:65], 1.0)

    # resident x_q^T [128, cch*S] bf16 (32KB/partition)
    xq_res = persist.tile([128, CCH * S], BF16, tag="xq_res")

    def load_xq_res_tb(tb, halves=1):
        t_sl = slice(512 * tb, 512 * (tb + 1))
        hc = CCH // halves
        for i in range(halves):
            c_sl = slice(hc * i, hc * (i + 1))
            nc.sync.dma_start(
                xq_res[:].rearrange("p (c t) -> p c t", c=CCH)[:, c_sl, t_sl],
                xqT[:].rearrange("(c p) t -> p c t", p=128)[:, c_sl, t_sl],
            )

    def load_x_stage(xT, key, tb, halves=1):
        xt = xpool.tile(
            [128, CCH * 512], BF16, tag=f"x{key}", name=f"x{key}",
            bufs=4 if key == "k" else 2,
        )
        hc = CCH // halves
        for i in range(halves):
            nc.sync.dma_start(
                xt[:].rearrange("p (c t) -> p c t", c=CCH)[:, hc * i:hc * (i + 1)],
                xT[:].rearrange("(c p) t -> p c t", p=128)[
                    :, hc * i : hc * (i + 1), 512 * tb : 512 * (tb + 1)
                ],
            )
        return xt

    # ---- projection units (8 matmuls + 1 DVE drain each) ----
    def kq_unit(xt, wkey, dst, bias_sb, p, tb):
        ps = psum.tile([128, 512], F32, tag="mmp", bufs=2, name="ps")
        for c in range(CCH):
            nc.tensor.matmul(
                ps[:, 0:512],
                (w_sb[wkey][:, c * DG + 128 * p : c * DG + 128 * (p + 1)]),
                (xt[:, 512 * c : 512 * (c + 1)]),
                start=(c == 0),
                stop=(c == CCH - 1),
            )
        nc.vector.tensor_scalar_add(
            dst[p][:, 512 * tb : 512 * (tb + 1)],
            ps[:, 0:512],
            bias_sb[:, p : p + 1],
        )

    def q_unit(p, tb):
        ps = psum.tile([128, 512], F32, tag="mmp", bufs=2, name="ps")
        for c in range(CCH):
            nc.tensor.matmul(
                ps[:, 0:512],
                (w_sb["q"][:, c * DG + 128 * p : c * DG + 128 * (p + 1)]),
                (xq_res[:, c * S + 512 * tb : c * S + 512 * (tb + 1)]),
                start=(c == 0),
                stop=(c == CCH - 1),
            )
        nc.vector.tensor_scalar_add(
            qt_sb[p][:, 512 * tb : 512 * (tb + 1)],
            ps[:, 0:512],
            bq_sb[:, p : p + 1],
        )

    def v_unit(xt, tb, j):
        c = 4 * tb + j
        ps = psum.tile([128, 512], F32, tag="mmp", bufs=2, name="ps")
        for cc in range(CCH):
            nc.tensor.matmul(
                ps[:, 0:512],
                (xt[:, 512 * cc + 128 * j : 512 * cc + 128 * (j + 1)]),
                (w_sb["v"][:, cc * DG : (cc + 1) * DG]),
                start=(cc == 0),
                stop=(cc == CCH - 1),
            )
        nc.vector.tensor_add(
            v_r[:, :, 2 * c : 2 * c + 2, 0:64],
            ps[:, 0:512].rearrange("t (p h d) -> t p h d", p=NPAIR, h=2),
            bv_rep[:].rearrange("t (p h d) -> t p h d", p=NPAIR, h=2),
        )

    # ---- exp paths ----
    def emit_exp_scalar(sc, c):
        pt = work.tile([128, 1024], BF16, tag="pt", bufs=PT_BUFS)
        nc.scalar.activation(
            pt[:], sc[:], Exp, bias=mask_sb[:, c : c + 1], scale=0.125
        )
        return pt

    # Split-chunk exp: ScalarE takes head 0, the DVE takes head 1 via the
    # pipelined int32-convert (frees its sc half fast) + custom fixup two
    # slots later.
    def emit_exp_split1(sc, c):
        pt = work.tile([128, 1024], BF16, tag="pt", bufs=PT_BUFS)
        ti = work.tile([128, 1024], I32, tag="ti", bufs=2)
        nc.vector.tensor_scalar(ti[:], sc[:], dvea_sb[:, 0:1],
                                dveb_sb[:, c : c + 1],
                                AluOpType.mult, AluOpType.add)
        return pt, ti

    def emit_exp_split1_mmp(schalves, c):
        pt = work.tile([128, 1024], BF16, tag="pt", bufs=PT_BUFS)
        ti = work.tile([128, 1024], I32, tag="ti", bufs=2)
        for h in (0, 1):
            hs = slice(512 * h, 512 * (h + 1))
            nc.vector.tensor_scalar(ti[:, hs], schalves[h][:, 0:512],
                                    dvea_sb[:, 0:1], dveb_sb[:, c : c + 1],
                                    AluOpType.mult, AluOpType.add)
        return pt, ti

    def emit_exp_split2(ti, pt):
        nc.vector._custom_dve(exp2_op, out=pt[:],
                              in0=ti[:].bitcast(F32),
                              in1=t3c_sb[:], s0=mand_sb, s1=morr_sb,
                              imm2=float(EXP_S3))

    # ---- flat attention pipeline over slots ----
    NBLK = NPAIR * QB

    ctx_of = {}
    started = {}

    def emit_pv(blk, c, pt, stop):
        p, qb = blk // QB, blk % QB
        if blk not in ctx_of:
            ctx_of[blk] = [
                psum.tile([128, 260], F32, tag=f"ctx{h}", name=f"ctx{h}")
                for h in range(2)
            ]
            started[blk] = [False, False]
        for h in range(2):
            first = not started[blk][h]
            started[blk][h] = True
            for qc in range(4):
                nc.tensor.matmul(
                    ctx_of[blk][h][:, 65 * qc : 65 * qc + 65],
                    (pt[:, 512 * h + 128 * qc : 512 * h + 128 * (qc + 1)]),
                    (v_r[:, p, 2 * c + h, :]),
                    start=(first and qc == 0),
                    stop=(stop and qc == 3),
                    skip_group_check=True,
                )

    def emit_drain(blk):
        p, qb = blk // QB, blk % QB
        for h in range(2):
            cs = work.tile([128, 260], F32, tag="cs", name="cs", bufs=3)
            nc.vector.tensor_scalar_add(cs[:], ctx_of[blk][h][:], 0.0)
            r0 = ((p * 2 + h) * QB + qb) * 128
            nc.sync.dma_start(out[r0 : r0 + 128, :], cs[:])
        del ctx_of[blk]
        del started[blk]

    # ---- preamble: warmup + the critical-path DMAs + pair-0 tb0 units ----
    # DMA serializes at ~2.9us/MB on the DMA fabric; the SP HWDGE queue is
    # in-order, so the queue order is exactly the need order: wk-p0, xk0,
    # wq-p0, xq0, cst unblock the first scores; the rest queues behind in
    # first-use order. The xv tb2/tb3 re-stages (which must WAIT on V-unit
    # consumers) go on the Pool engine's SWDGE queue so their waits cannot
    # head-block the SP queue.
    warmup(WARMUP_N)
    load_w_pair0("k")
    xk = [load_x_stage(xkT, "k", 0, halves=2)]
    load_w_pair0("q")
    load_xq_res_tb(0, halves=2)
    nc.sync.dma_start(cst_sb[:], cst[:])
    nc.sync.dma_start(bv_rep[:], bv[:])
    kq_unit(xk[0], "k", kt_sb, bk_sb, 0, 0)
    q_unit(0, 0)
    xk.append(load_x_stage(xkT, "k", 1))
    xk.append(load_x_stage(xkT, "k", 2))
    load_w("v")
    xv_t = {0: load_x_stage(xvT, "v", 0)}
    load_xq_res_tb(1)
    xk.append(load_x_stage(xkT, "k", 3))
    xv_t[1] = load_x_stage(xvT, "v", 1)
    load_xq_res_tb(2)
    load_w_rest("k")
    load_w_rest("q")
    load_xq_res_tb(3)

    # ---- filler schedule: slot -> [callables] ----
    filler = {}

    def add_filler(slot, fn):
        filler.setdefault(slot, []).append(fn)

    # dependency-free warmup batches bridge the early DMA-wait region
    # without blocking the in-order PE queue
    add_filler(1, lambda: warmup(5))
    add_filler(3, lambda: warmup(5))

    # K p0 tb1-3 as soon as their x lands
    add_filler(0, lambda: kq_unit(xk[1], "k", kt_sb, bk_sb, 0, 1))
    add_filler(4, lambda: kq_unit(xk[2], "k", kt_sb, bk_sb, 0, 2))
    add_filler(8, lambda: kq_unit(xk[3], "k", kt_sb, bk_sb, 0, 3))

    # V: one unit per slot for slots 5..20; xv tb2/3 re-staged via Pool DMA
    def stage_xv(tb):
        xt = xpool.tile([128, CCH * 512], BF16, tag="xv", name="xv", bufs=2)
        nc.gpsimd.dma_start(
            xt[:].rearrange("p (c t) -> p c t", c=CCH),
            xvT[:].rearrange("(c p) t -> p c t", p=128)[
                :, :, 512 * tb : 512 * (tb + 1)
            ],
        )
        xv_t[tb] = xt

    add_filler(11, lambda: stage_xv(2))
    add_filler(15, lambda: stage_xv(3))
    for tb in range(TB):
        for j in range(4):
            add_filler(5 + 4 * tb + j,
                       lambda tb=tb, j=j: v_unit(xv_t[tb], tb, j))
    # Q(p, qb) spread to just ahead of block (p,qb) at slot 64p+16qb
    for p in range(NPAIR):
        for qb in range(QB):
            if p == 0 and qb == 0:
                continue
            slot = 64 * p + 16 * qb - 7
            if p == 3:
                slot = 150 + 3 * qb   # keep blocks 10-15 filler-free
            add_filler(slot, lambda p=p, qb=qb: q_unit(p, qb))
    # later pairs: K units (x_k stays resident) ahead of slot 64*p
    for p in range(1, NPAIR):
        base = 64 * p if p < 3 else 156
        for tb in range(TB):
            add_filler(base - 20 + 4 * tb,
                       lambda p=p, tb=tb: kq_unit(xk[tb], "k", kt_sb,
                                                  bk_sb, p, tb))

    # ---- the flat stream ----
    pend = []     # (due_slot, blk, c, pt)
    stage2q = []  # (due_slot, ti, pt)
    slot = 0
    for blk in range(NBLK):
        p, qb = blk // QB, blk % QB
        q_sl = slice(512 * qb, 512 * (qb + 1))
        dve_cs = _dve_positions(blk)
        for c in range(TCH):
            kt_sl = slice(128 * c, 128 * (c + 1))
            use_mmp = c in dve_cs and blk >= 10
            if use_mmp:
                schalves = [
                    psum.tile([128, 512], F32, tag="mmp", bufs=2, name="ps")
                    for _ in range(2)
                ]
            else:
                sc = psum.tile([128, 1024], F32, tag="mm", bufs=2)
            for h in (0, 1):
                hp = slice(64 * h, 64 * (h + 1))
                nc.tensor.matmul(
                    schalves[h][:, 0:512] if use_mmp
                    else sc[:, 512 * h : 512 * (h + 1)],
                    (kt_sb[p][hp, kt_sl]),
                    (qt_sb[p][hp, q_sl]),
                    start=True,
                    stop=True,
                )
            if c in dve_cs:
                if use_mmp:
                    pt, ti = emit_exp_split1_mmp(schalves, c)
                else:
                    pt, ti = emit_exp_split1(sc, c)
                stage2q.append((slot + 2, ti, pt))
                lag = max(_blk_lag(blk), PV_LAG_DVE)
            else:
                pt = emit_exp_scalar(sc, c)
                lag = _blk_lag(blk)
            pend.append((slot + lag, blk, c, pt))
            pend.sort(key=lambda e: e[0])
            while stage2q and stage2q[0][0] <= slot:
                _, ti2, pt2 = stage2q.pop(0)
                emit_exp_split2(ti2, pt2)
            while pend and pend[0][0] <= slot:
                _, b2, c2, pt2 = pend.pop(0)
                done_emitting = b2 < blk or (b2 == blk and c == TCH - 1)
                stop = done_emitting and not any(e[1] == b2 for e in pend)
                emit_pv(b2, c2, pt2, stop)
                if stop:
                    emit_drain(b2)
            for fn in filler.pop(slot, []):
                fn()
            slot += 1
    while stage2q:
        _, ti2, pt2 = stage2q.pop(0)
        emit_exp_split2(ti2, pt2)
    while pend:
        _, b2, c2, pt2 = pend.pop(0)
        stop = not any(e[1] == b2 for e in pend)
        emit_pv(b2, c2, pt2, stop)
        if stop:
            emit_drain(b2)
    for sl in sorted(filler):
        for fn in filler.pop(sl):
            fn()

    pools.close()


def make_in_maps(x_q, x_k, x_v, att_mask, W_q, b_q, W_k, b_k, W_v, b_v):
    import ml_dtypes

    f = np.float32
    bf = ml_dtypes.bfloat16
    x_q, x_k, x_v = (np.asarray(a, f) for a in (x_q, x_k, x_v))
    att_mask = np.asarray(att_mask, f)
    W_q, W_k, W_v = (np.asarray(a, f) for a in (W_q, W_k, W_v))
    b_q, b_k, b_v = (np.asarray(a, f) for a in (b_q, b_k, b_v))

    in_maps = []
    for core in range(NCORES):
        b, g = divmod(core, TP)
        fsl = slice(DG * g, DG * (g + 1))
        mask_col = np.ascontiguousarray(
            att_mask[b, 0, 0].reshape(TCH, 128).T)           # [128, TCH]
        dve_b = (2.0 ** 23 * EXP_B0
                 + mask_col.astype(np.float64) * (2.0 ** 23 * LOG2E)
                 ).astype(np.float32)
        dve_a = np.full((128, 1), EXP_A, np.float32)
        m_and = np.full((128, 1), 0x007FFFFF, np.int32).view(np.float32)
        m_orr = np.full((128, 1), 0x3F800000, np.int32).view(np.float32)
        in_maps.append(
            {
                "xqT": np.ascontiguousarray(x_q[b].T.astype(bf)),
                "xkT": np.ascontiguousarray(x_k[b].T.astype(bf)),
                "xvT": np.ascontiguousarray(x_v[b].T.astype(bf)),
                "wqT": np.ascontiguousarray(W_q[fsl, :].T.astype(bf)),
                "wkT": np.ascontiguousarray(W_k[fsl, :].T.astype(bf)),
                "wvT": np.ascontiguousarray(W_v[fsl, :].T.astype(bf)),
                "cst": np.ascontiguousarray(
                    np.concatenate(
                        [
                            b_q[fsl].reshape(NPAIR, 128).T,
                            b_k[fsl].reshape(NPAIR, 128).T,
                            mask_col,
                            dve_b,
                            dve_a,
                            m_and,
                            m_orr,
                        ],
                        axis=1,
                    )
                ),
                "bv": np.ascontiguousarray(
                    np.broadcast_to(b_v[fsl].reshape(1, DG), (128, DG))
                ).astype(bf),
            }
        )
    return in_maps


def kernel(x_q, x_k, x_v, att_mask, W_q, b_q, W_k, b_k, W_v, b_v):
    global _CACHED
    if _CACHED is None:
        _CACHED = _build_core_program()
    nc = _CACHED

    in_maps = make_in_maps(
        x_q, x_k, x_v, att_mask, W_q, b_q, W_k, b_k, W_v, b_v
    )

    import os

    global LAST_RESULTS
    trace = TRACE or os.environ.get("BASS_KERNEL_TRACE", "") == "1"
    try:
        res = run_bass_kernel_spmd(nc, in_maps, list(range(NCORES)), trace=trace)
    except Exception:
        if not trace:
            raise
        res = run_bass_kernel_spmd(nc, in_maps, list(range(NCORES)))
    LAST_RESULTS = res

    full = np.empty((B, S, H), np.float32)
    for core in range(NCORES):
        b, g = divmod(core, TP)
        r = res.results[core]["out"].reshape(NPAIR, 2, QB, 128, 4, 65)
        ctx = r[..., 0:64] / r[..., 64:65]          # [p, h, qb, i, qc, d]
        full[b, :, DG * g : DG * (g + 1)] = (
            ctx.transpose(2, 4, 3, 0, 1, 5).reshape(S, DG)
        )
    return full
